# revision 1
# baseline (speedup 1.0000x reference)
"""Content-guided attention kernel for Trainium2, 8 NeuronCores SPMD.

Sharding: 8 cores = (batch b in {0,1}) x (query-chunk qc in {0..3}).
Each core computes 1024 query positions of batch b end-to-end:
q/k/vT projections, 8-head attention over all 3072 keys, o-projection,
residual and LayerNorm.  No collectives needed; host splits/concats.

Per-core layout highlights:
 - scores computed transposed S^T[kpos, qpos] so softmax sum folds into the
   attn@V matmul via a ones-column appended to V^T (no partition reductions)
 - head_dim=32 scores matmuls are packed 4-at-a-time into the PE's 32-row
   groups via tile_position (4x concurrency at K=32)
 - exp split between ScalarE (exact table exp) and VectorE (Schraudolph
   bit-trick exp, ~3% elementwise, ~1.5e-4 end-to-end after softmax
   cancellation + residual/LN dilution)
 - LayerNorm rstd computed as exp(-0.5*ln(var+eps)) to stay inside the
   single natural_log_exp ACT table set (no table switch thrash)
"""

import numpy as np

C = 256
NH = 8
D = 32
NQ_CORE = 1024
NK = 3072
N_CORES = 8
SCALE = float(D) ** -0.5

# Schraudolph exp constants (validated vs reference offline: 3.0% max elem
# rel err on the observed score range; 1.5e-4 absmax on the final output).
_SCHR_A = float(np.float32(SCALE * (1 << 23) / np.log(2.0)))
_SCHR_B = float(np.float32(127.0 * (1 << 23) - 365000.0))

# every 3rd exp slot goes to the vector engine to offload the ACT bottleneck
def _use_dve_exp(slot: int) -> bool:
    return slot % 3 == 2


def _apply_walrus_wait_patch():
    """This walrus build accepts only ONE sync-wait per instruction; split
    extra waits onto single-wait NoOps inserted before the instruction
    (same engine, same block => per-engine program order preserved)."""
    import orjson
    import concourse.bass_utils as bass_utils
    import concourse.bass2jax as bass2jax

    if getattr(bass_utils, "_ant_wait_split_patch", False):
        return
    bass_utils._ant_wait_split_patch = True
    counter = [0]

    def _split_waits(bir_bytes: bytes) -> bytes:
        d = orjson.loads(bir_bytes)
        changed = False

        def process_blocks(blocks):
            nonlocal changed
            for b in blocks:
                insts = b.get("instructions")
                if insts:
                    new = []
                    for ins in insts:
                        si = ins.get("sync_info")
                        waits = si.get("on_wait") if si else None
                        if waits and len(waits) > 1:
                            changed = True
                            for w in waits[:-1]:
                                counter[0] += 1
                                new.append({
                                    "debug": ins.get("debug", 0),
                                    "engine": ins["engine"],
                                    "ins": [],
                                    "outs": [],
                                    "name": f"antwsplit-{counter[0]}",
                                    "opcode": "NoOp",
                                    "sync_info": {"on_wait": [w], "on_update": []},
                                })
                            si["on_wait"] = [waits[-1]]
                        new.append(ins)
                    b["instructions"] = new
                if b.get("blocks"):
                    process_blocks(b["blocks"])

        for f in d.get("functions", []):
            process_blocks(f.get("blocks", []))
        return orjson.dumps(d) if changed else bir_bytes

    orig = bass_utils.compile_bir_kernel

    def compile_bir_kernel(bir, tmpdir, neff_name="file.neff", **kw):
        if isinstance(bir, (bytes, bytearray)):
            bir = _split_waits(bytes(bir))
        elif isinstance(bir, str):
            bir = _split_waits(bir.encode()).decode()
        return orig(bir, tmpdir, neff_name=neff_name, **kw)

    bass_utils.compile_bir_kernel = compile_bir_kernel
    bass2jax.compile_bir_kernel = compile_bir_kernel


def build_program():
    import concourse.bass as bass
    import concourse.tile as tile
    from concourse import mybir

    f32 = mybir.dt.float32
    i32 = mybir.dt.int32
    Alu = mybir.AluOpType
    Act = mybir.ActivationFunctionType

    nc = bass.Bass()

    x_d = nc.dram_tensor("x", [C, NQ_CORE], f32, kind="ExternalInput")
    kv_d = nc.dram_tensor("kv", [C, NK], f32, kind="ExternalInput")
    qwT_d = nc.dram_tensor("qwT", [C, C], f32, kind="ExternalInput")
    kwT_d = nc.dram_tensor("kwT", [C, C], f32, kind="ExternalInput")
    vwT_d = nc.dram_tensor("vwT", [C, C], f32, kind="ExternalInput")
    owT_d = nc.dram_tensor("owT", [C, C], f32, kind="ExternalInput")
    ident_d = nc.dram_tensor("ident", [C, C], f32, kind="ExternalInput")
    qb_d = nc.dram_tensor("qb2", [1, C], f32, kind="ExternalInput")
    kb_d = nc.dram_tensor("kb2", [1, C], f32, kind="ExternalInput")
    vb_d = nc.dram_tensor("vb2", [1, C], f32, kind="ExternalInput")
    ob_d = nc.dram_tensor("ob2", [1, C], f32, kind="ExternalInput")
    lnw_d = nc.dram_tensor("lnw2", [1, C], f32, kind="ExternalInput")
    lnb_d = nc.dram_tensor("lnb2", [1, C], f32, kind="ExternalInput")
    y_d = nc.dram_tensor("y", [NQ_CORE, C], f32, kind="ExternalOutput")

    def bcast_part(ap, n):
        # partition-stride-0 view: replicate one partition row across n
        # (DRAM sources only; SBUF partition dims need nonzero step)
        return bass.AP(tensor=ap.tensor, offset=ap.offset,
                       ap=[[0, n]] + [list(a) for a in ap.ap[1:]])

    def bcast_sbuf_row(ap, n):
        # SBUF [1, F] row -> [n, F] DMA source: keep the 1-partition dim,
        # replicate via a step-0 free dim (legal for DMA reads)
        return bass.AP(tensor=ap.tensor, offset=ap.offset,
                       ap=[list(ap.ap[0]), [0, n]] + [list(a) for a in ap.ap[1:]])

    from contextlib import ExitStack
    with tile.TileContext(nc) as tc, ExitStack() as ctx:
            consts = ctx.enter_context(tc.tile_pool(name="consts", bufs=1))
            data = ctx.enter_context(tc.tile_pool(name="data", bufs=1))
            acts = ctx.enter_context(tc.tile_pool(name="acts", bufs=1))
            # ---- constants ----
            w_sb = {}
            for nm, dt_ in (("qwT", qwT_d), ("kwT", kwT_d), ("vwT", vwT_d),
                            ("owT", owT_d), ("ident", ident_d)):
                t = consts.tile([128, 2, C], f32, tag=f"w_{nm}")
                nc.sync.dma_start(out=t, in_=dt_.rearrange("(a p) c -> p a c", p=128))
                w_sb[nm] = t
            qb_row = consts.tile([1, C], f32, tag="qb_row")
            kb_row = consts.tile([1, C], f32, tag="kb_row")
            vb_row = consts.tile([1, C], f32, tag="vb_row")
            ob_row = consts.tile([1, C], f32, tag="ob_row")
            for t, dt_ in ((qb_row, qb_d), (kb_row, kb_d), (vb_row, vb_d), (ob_row, ob_d)):
                nc.sync.dma_start(out=t, in_=dt_[:])
            lnw_bc = consts.tile([128, C], f32, tag="lnw_bc")
            lnb_bc = consts.tile([128, C], f32, tag="lnb_bc")
            nc.sync.dma_start(out=lnw_bc, in_=bcast_part(lnw_d[:], 128))
            nc.sync.dma_start(out=lnb_bc, in_=bcast_part(lnb_d[:], 128))
            ones_row = consts.tile([1, 512], f32, tag="ones_row")
            nc.vector.memset(ones_row, 1.0)
            eps_col = consts.tile([128, 1], f32, tag="eps_col")
            nc.vector.memset(eps_col, 1e-5)

            # ---- input activations ----
            x_sb = data.tile([128, 2, NQ_CORE], f32, tag="x_sb")
            nc.sync.dma_start(out=x_sb, in_=x_d.rearrange("(a p) n -> p a n", p=128))
            kv_sb = data.tile([128, 2, NK], f32, tag="kv_sb")
            nc.sync.dma_start(out=kv_sb, in_=kv_d.rearrange("(a p) n -> p a n", p=128))

            q_sb = acts.tile([128, 2, NQ_CORE], f32, tag="q_sb")
            k_sb = acts.tile([128, 2, NK], f32, tag="k_sb")
            vT_aug = acts.tile([128, 24, NH, D + 1], f32, tag="vT_aug")
            nc.vector.memset(vT_aug[:, :, :, D:D + 1], 1.0)

            # ---- projections ----
            with tc.tile_pool(name="proj_ps", bufs=4, space="PSUM") as proj_ps:
                # q = qw @ x + qb   (chunks of output channels x 512 cols)
                for mc in range(2):
                    for nb in range(2):
                        ps = proj_ps.tile([128, 512], f32, tag="proj")
                        for kc2 in range(2):
                            nc.tensor.matmul(
                                ps, lhsT=w_sb["qwT"][:, kc2, mc * 128:(mc + 1) * 128],
                                rhs=x_sb[:, kc2, nb * 512:(nb + 1) * 512],
                                start=(kc2 == 0), stop=False)
                        nc.tensor.matmul(
                            ps, lhsT=qb_row[:, mc * 128:(mc + 1) * 128],
                            rhs=ones_row[:, 0:512], start=False, stop=True)
                        nc.vector.tensor_copy(q_sb[:, mc, nb * 512:(nb + 1) * 512], ps)
                # k = kw @ kv + kb
                for mc in range(2):
                    for nb in range(6):
                        ps = proj_ps.tile([128, 512], f32, tag="proj")
                        for kc2 in range(2):
                            nc.tensor.matmul(
                                ps, lhsT=w_sb["kwT"][:, kc2, mc * 128:(mc + 1) * 128],
                                rhs=kv_sb[:, kc2, nb * 512:(nb + 1) * 512],
                                start=(kc2 == 0), stop=False)
                        nc.tensor.matmul(
                            ps, lhsT=kb_row[:, mc * 128:(mc + 1) * 128],
                            rhs=ones_row[:, 0:512], start=False, stop=True)
                        nc.vector.tensor_copy(k_sb[:, mc, nb * 512:(nb + 1) * 512], ps)
                # vT[n, c] = (kv^T @ vw^T)[n, c] + vb[c], written per-head with
                # a ones column appended (softmax denominator trick)
                for nn in range(24):
                    ps = proj_ps.tile([128, C], f32, tag="proj")
                    for kc2 in range(2):
                        nc.tensor.matmul(
                            ps, lhsT=kv_sb[:, kc2, nn * 128:(nn + 1) * 128],
                            rhs=w_sb["vwT"][:, kc2, :], start=(kc2 == 0), stop=False)
                    nc.tensor.matmul(ps, lhsT=ones_row[0:1, 0:128], rhs=vb_row[:],
                                     start=False, stop=True)
                    nc.vector.tensor_copy(
                        vT_aug[:, nn, :, 0:D],
                        ps.rearrange("p (h e) -> p h e", h=NH))

            # ---- attention + o-proj + LN ----
            with tc.tile_pool(name="s_ps", bufs=3, space="PSUM") as s_pool, \
                 tc.tile_pool(name="o_ps", bufs=1, space="PSUM") as o_pool, \
                 tc.tile_pool(name="exps", bufs=3) as exp_pool, \
                 tc.tile_pool(name="tails", bufs=2) as tails, \
                 tc.tile_pool(name="norms", bufs=2) as norms, \
                 tc.tile_pool(name="fins", bufs=2) as fins:
                for qb in range(2):
                    # onrm[hg] accumulates the 4 normalized heads of chunk hg
                    onrm0 = norms.tile([128, 512], f32, tag="onrm0")
                    onrm1 = norms.tile([128, 512], f32, tag="onrm1")
                    onrm_tiles = [onrm0, onrm1]
                    for hp in range(4):          # head pairs
                        hg, sub = hp // 2, hp % 2
                        # concurrent tile_position row-groups must land in
                        # SEPARATE psum banks (same-bank pairs fault the PE)
                        po = o_pool.tile([D + 1, 2, 512], f32, tag="opo")
                        for kc in range(24):
                            ps = s_pool.tile([128, 2, 512], f32, tag="S")
                            for j in range(2):
                                pof = 64 * sub + 32 * j
                                nc.tensor.matmul(
                                    ps[:, j, :],
                                    lhsT=k_sb[pof:pof + 32, hg, kc * 128:(kc + 1) * 128],
                                    rhs=q_sb[pof:pof + 32, hg, qb * 512:(qb + 1) * 512],
                                    start=True, stop=True, tile_position=(pof, 0))
                            slot = (qb * 4 + hp) * 24 + kc
                            if _use_dve_exp(slot):
                                es_i = exp_pool.tile([128, 2, 512], i32, tag="exp")
                                nc.vector.tensor_scalar(
                                    out=es_i, in0=ps, scalar1=_SCHR_A, scalar2=_SCHR_B,
                                    op0=Alu.mult, op1=Alu.add)
                                es = es_i.bitcast(f32)
                            else:
                                es = exp_pool.tile([128, 2, 512], f32, tag="exp")
                                nc.scalar.activation(es, ps, Act.Exp, scale=SCALE)
                            for j in range(2):
                                nc.tensor.matmul(
                                    po[:, j, :],
                                    lhsT=vT_aug[:, kc, hp * 2 + j, :],
                                    rhs=es[:, j, :],
                                    start=(kc == 0), stop=(kc == 23))
                        # tail: numerators + softmax denominators
                        raw = tails.tile([D + 1, 2, 512], f32, tag="raw")
                        nc.vector.tensor_copy(raw, po)
                        # denominators live on ONE partition row; iterative
                        # reciprocal is 8cyc/elem/lane, so spread the 1024
                        # values over 32 partitions via DMA, recip, pack back
                        dp = tails.tile([32, 32], f32, tag="dp")
                        nc.sync.dma_start(
                            out=dp, in_=raw[D:D + 1, :, :].rearrange("p a q -> p (a q)"))
                        rp = tails.tile([32, 32], f32, tag="rp")
                        nc.vector.reciprocal(rp, dp)
                        rec = tails.tile([1, 2, 512], f32, tag="rec")
                        nc.sync.dma_start(
                            out=rec.rearrange("p a q -> p (a q)"), in_=rp)
                        oin = tails.tile([128, 512], f32, tag="oin")
                        rbc = tails.tile([128, 512], f32, tag="rbc")
                        for j in range(2):
                            pof = 64 * sub + 32 * j
                            nc.sync.dma_start(out=oin[pof:pof + 32, :],
                                              in_=raw[0:D, j, :])
                            nc.sync.dma_start(out=rbc[pof:pof + 32, :],
                                              in_=bcast_sbuf_row(rec[0:1, j, :], 32))
                        nc.vector.tensor_mul(
                            onrm_tiles[hg][64 * sub:64 * sub + 64, :],
                            oin[64 * sub:64 * sub + 64, :],
                            rbc[64 * sub:64 * sub + 64, :])
                    # o-projection + residual + bias + LayerNorm per 128 queries
                    for qc2 in range(4):
                        qoff = qb * 512 + qc2 * 128
                        pso = s_pool.tile([128, C], f32, tag="S")
                        for hgc in range(2):
                            nc.tensor.matmul(
                                pso, lhsT=onrm_tiles[hgc][:, qc2 * 128:(qc2 + 1) * 128],
                                rhs=w_sb["owT"][:, hgc, :],
                                start=(hgc == 0), stop=False)
                        for cc in range(2):
                            nc.tensor.matmul(
                                pso, lhsT=x_sb[:, cc, qoff:qoff + 128],
                                rhs=w_sb["ident"][:, cc, :], start=False, stop=False)
                        nc.tensor.matmul(pso, lhsT=ones_row[0:1, 0:128], rhs=ob_row[:],
                                         start=False, stop=True)
                        stats = fins.tile([128, 6], f32, tag="stats")
                        nc.vector.bn_stats(stats, pso)
                        mv = fins.tile([128, 2], f32, tag="mv")
                        nc.vector.bn_aggr(mv, stats)
                        # rstd = exp(-0.5*ln(var+eps)): stays in the same ACT
                        # table set as the softmax exp (no table reload)
                        lnv = fins.tile([128, 1], f32, tag="lnv")
                        nc.scalar.activation(lnv, mv[:, 1:2], Act.Ln, bias=eps_col[:, 0:1])
                        rstd = fins.tile([128, 1], f32, tag="rstd")
                        nc.scalar.activation(rstd, lnv, Act.Exp, scale=-0.5)
                        t1 = fins.tile([128, C], f32, tag="t1")
                        nc.vector.tensor_scalar(
                            out=t1, in0=pso, scalar1=mv[:, 0:1], scalar2=rstd,
                            op0=Alu.subtract, op1=Alu.mult)
                        t2 = fins.tile([128, C], f32, tag="t2")
                        nc.vector.tensor_mul(t2, t1, lnw_bc)
                        t3 = fins.tile([128, C], f32, tag="t3")
                        nc.vector.tensor_add(t3, t2, lnb_bc)
                        nc.sync.dma_start(out=y_d[qoff:qoff + 128, :], in_=t3)
    return nc


_CACHE = {}


def _get_program():
    if "nc" not in _CACHE:
        _apply_walrus_wait_patch()
        _CACHE["nc"] = build_program()
    return _CACHE["nc"]


def _make_in_maps(inputs):
    s3 = np.ascontiguousarray(np.asarray(inputs["s3"], dtype=np.float32))
    s4 = np.ascontiguousarray(np.asarray(inputs["s4"], dtype=np.float32))
    s5 = np.ascontiguousarray(np.asarray(inputs["s5"], dtype=np.float32))
    B = s3.shape[0]
    wts = {}
    for nm in ("qw", "kw", "vw", "ow"):
        wts[nm + "T"] = np.ascontiguousarray(np.asarray(inputs[nm], dtype=np.float32).T)
    ident = np.eye(C, dtype=np.float32)
    rows = {}
    for nm in ("qb", "kb", "vb", "ob", "ln_w", "ln_b"):
        rows[nm] = np.ascontiguousarray(
            np.asarray(inputs[nm], dtype=np.float32).reshape(1, C))
    in_maps = []
    for core in range(N_CORES):
        b, qc = core // 4, core % 4
        x = np.ascontiguousarray(
            s3[b].reshape(C, -1)[:, qc * NQ_CORE:(qc + 1) * NQ_CORE])
        kv = np.ascontiguousarray(np.concatenate(
            [s4[b].reshape(C, -1), s5[b].reshape(C, -1)], axis=1))
        in_maps.append({
            "x": x, "kv": kv,
            "qwT": wts["qwT"], "kwT": wts["kwT"], "vwT": wts["vwT"],
            "owT": wts["owT"], "ident": ident,
            "qb2": rows["qb"], "kb2": rows["kb"], "vb2": rows["vb"],
            "ob2": rows["ob"], "lnw2": rows["ln_w"], "lnb2": rows["ln_b"],
        })
    return in_maps


def _assemble(results, like):
    B, _, H, W = 2, C, 64, 64
    out = np.empty((B, C, H * W), dtype=np.float32)
    for core in range(N_CORES):
        b, qc = core // 4, core % 4
        out[b, :, qc * NQ_CORE:(qc + 1) * NQ_CORE] = results[core]["y"].T
    return out.reshape(B, C, H, W)


def kernel(**inputs):
    from concourse import bass2jax
    nc = _get_program()
    in_maps = _make_in_maps(inputs)
    results = bass2jax.run_bass_via_pjrt(nc, in_maps, n_cores=N_CORES)
    return _assemble(results, inputs["s3"])



# revision 4
# speedup vs baseline: 2.4638x; 2.4638x over previous
"""Content-guided attention kernel for Trainium2, 8 NeuronCores SPMD.

Sharding: 8 cores = (batch b in {0,1}) x (query-chunk qc in {0..3}).
Each core computes 1024 query positions of batch b end-to-end:
q/k/vT projections, 8-head attention over all 3072 keys, o-projection,
residual and LayerNorm.  No collectives needed; host splits/concats.

Per-core layout highlights:
 - all matmul operands in fp16 (1 cycle/row on the PE vs 4 for fp32;
   PSUM accumulation stays fp32)
 - scores computed transposed S^T[kpos, qpos] so softmax sum folds into the
   attn@V matmul via a ones-column appended to V^T (no partition reductions)
 - head_dim=32 scores matmuls are packed 4-at-a-time into the PE's 32-row
   groups via tile_position (4x concurrency at K=32)
 - attn@V matmuls col-packed 2-at-a-time (heads at output partitions 0 and
   64 of one PSUM bank) so the M=33 lane waste halves
 - q/k biases folded into the PSUM->SBUF eviction tensor_scalar; v bias via
   tensor_tensor add against a partition-broadcast tile
 - exp split between ScalarE (table exp, fp16 out) and VectorE (Schraudolph
   bit-trick exp emitted as int16 fp16-bitpattern)
 - LayerNorm rstd computed as exp(-0.5*ln(var+eps)) to stay inside the
   single natural_log_exp ACT table set (no table switch thrash)
"""

import numpy as np

C = 256
NH = 8
D = 32
NQ_CORE = 1024
NK = 3072
N_CORES = 8
SCALE = float(D) ** -0.5

# Schraudolph exp constants for an fp16 bit-pattern target:
# bits16 = round(s * SCALE * 1024/ln2 + (15*1024 - 44.56))
_SCHR_A16 = float(np.float32(SCALE * 1024.0 / np.log(2.0)))
_SCHR_B16 = float(np.float32(15.0 * 1024.0 - 44.56))

# exp slots: 3 of every 5 on ScalarE (table exp), 2 of 5 on VectorE
def _use_dve_exp(slot: int) -> bool:
    return slot % 5 >= 3


def _apply_walrus_wait_patch():
    """This walrus build accepts only ONE sync-wait per instruction; split
    extra waits onto single-wait NoOps inserted before the instruction
    (same engine, same block => per-engine program order preserved)."""
    import orjson
    import concourse.bass_utils as bass_utils
    import concourse.bass2jax as bass2jax

    if getattr(bass_utils, "_ant_wait_split_patch", False):
        return
    bass_utils._ant_wait_split_patch = True
    counter = [0]

    def _split_waits(bir_bytes: bytes) -> bytes:
        d = orjson.loads(bir_bytes)
        changed = False

        def process_blocks(blocks):
            nonlocal changed
            for b in blocks:
                insts = b.get("instructions")
                if insts:
                    new = []
                    for ins in insts:
                        si = ins.get("sync_info")
                        waits = si.get("on_wait") if si else None
                        if waits and len(waits) > 1:
                            changed = True
                            for w in waits[:-1]:
                                counter[0] += 1
                                new.append({
                                    "debug": ins.get("debug", 0),
                                    "engine": ins["engine"],
                                    "ins": [],
                                    "outs": [],
                                    "name": f"antwsplit-{counter[0]}",
                                    "opcode": "NoOp",
                                    "sync_info": {"on_wait": [w], "on_update": []},
                                })
                            si["on_wait"] = [waits[-1]]
                        new.append(ins)
                    b["instructions"] = new
                if b.get("blocks"):
                    process_blocks(b["blocks"])

        for f in d.get("functions", []):
            process_blocks(f.get("blocks", []))
        return orjson.dumps(d) if changed else bir_bytes

    orig = bass_utils.compile_bir_kernel

    def compile_bir_kernel(bir, tmpdir, neff_name="file.neff", **kw):
        if isinstance(bir, (bytes, bytearray)):
            bir = _split_waits(bytes(bir))
        elif isinstance(bir, str):
            bir = _split_waits(bir.encode()).decode()
        return orig(bir, tmpdir, neff_name=neff_name, **kw)

    bass_utils.compile_bir_kernel = compile_bir_kernel
    bass2jax.compile_bir_kernel = compile_bir_kernel


def build_program():
    import concourse.bass as bass
    import concourse.tile as tile
    from concourse import mybir

    f32 = mybir.dt.float32
    f16 = mybir.dt.float16
    i16 = mybir.dt.int16
    Alu = mybir.AluOpType
    Act = mybir.ActivationFunctionType

    nc = bass.Bass()

    x_d = nc.dram_tensor("x", [C, NQ_CORE], f16, kind="ExternalInput")
    kv_d = nc.dram_tensor("kv", [C, NK], f16, kind="ExternalInput")
    qwT_d = nc.dram_tensor("qwT", [C, C], f16, kind="ExternalInput")
    kwT_d = nc.dram_tensor("kwT", [C, C], f16, kind="ExternalInput")
    vwT_d = nc.dram_tensor("vwT", [C, C], f16, kind="ExternalInput")
    owT_d = nc.dram_tensor("owT", [C, C], f16, kind="ExternalInput")
    ident_d = nc.dram_tensor("ident", [C, C], f16, kind="ExternalInput")
    qb_d = nc.dram_tensor("qb2", [128, 2], f32, kind="ExternalInput")
    kb_d = nc.dram_tensor("kb2", [128, 2], f32, kind="ExternalInput")
    vb_d = nc.dram_tensor("vb2", [1, C], f32, kind="ExternalInput")
    ob_d = nc.dram_tensor("ob2", [1, C], f16, kind="ExternalInput")
    lnw_d = nc.dram_tensor("lnw2", [1, C], f32, kind="ExternalInput")
    lnb_d = nc.dram_tensor("lnb2", [1, C], f32, kind="ExternalInput")
    y_d = nc.dram_tensor("y", [NQ_CORE, C], f32, kind="ExternalOutput")

    def bcast_part(ap, n):
        # partition-stride-0 view: replicate one partition row across n
        # (DRAM sources only; SBUF partition dims need nonzero step)
        return bass.AP(tensor=ap.tensor, offset=ap.offset,
                       ap=[[0, n]] + [list(a) for a in ap.ap[1:]])

    def bcast_sbuf_row(ap, n):
        # SBUF [1, F] row -> [n, F] DMA source: keep the 1-partition dim,
        # replicate via a step-0 free dim (legal for DMA reads)
        return bass.AP(tensor=ap.tensor, offset=ap.offset,
                       ap=[list(ap.ap[0]), [0, n]] + [list(a) for a in ap.ap[1:]])

    from contextlib import ExitStack
    with tile.TileContext(nc) as tc, ExitStack() as ctx:
            consts = ctx.enter_context(tc.tile_pool(name="consts", bufs=1))
            data = ctx.enter_context(tc.tile_pool(name="data", bufs=1))
            acts = ctx.enter_context(tc.tile_pool(name="acts", bufs=1))
            # ---- constants ----
            w_sb = {}
            for nm, dt_ in (("qwT", qwT_d), ("kwT", kwT_d), ("vwT", vwT_d),
                            ("owT", owT_d), ("ident", ident_d)):
                t = consts.tile([128, 2, C], f16, tag=f"w_{nm}")
                nc.sync.dma_start(out=t, in_=dt_.rearrange("(a p) c -> p a c", p=128))
                w_sb[nm] = t
            qb_col = consts.tile([128, 2], f32, tag="qb_col")
            kb_col = consts.tile([128, 2], f32, tag="kb_col")
            nc.sync.dma_start(out=qb_col, in_=qb_d[:])
            nc.sync.dma_start(out=kb_col, in_=kb_d[:])
            ob_row = consts.tile([1, C], f16, tag="ob_row")
            nc.sync.dma_start(out=ob_row, in_=ob_d[:])
            vb_bc = consts.tile([128, C], f32, tag="vb_bc")
            nc.sync.dma_start(out=vb_bc, in_=bcast_part(vb_d[:], 128))
            lnw_bc = consts.tile([128, C], f32, tag="lnw_bc")
            lnb_bc = consts.tile([128, C], f32, tag="lnb_bc")
            nc.sync.dma_start(out=lnw_bc, in_=bcast_part(lnw_d[:], 128))
            nc.sync.dma_start(out=lnb_bc, in_=bcast_part(lnb_d[:], 128))
            ones_row = consts.tile([1, 512], f16, tag="ones_row")
            nc.vector.memset(ones_row, 1.0)
            eps_col = consts.tile([128, 1], f32, tag="eps_col")
            nc.vector.memset(eps_col, 1e-5)

            # ---- input activations ----
            x_sb = data.tile([128, 2, NQ_CORE], f16, tag="x_sb")
            nc.sync.dma_start(out=x_sb, in_=x_d.rearrange("(a p) n -> p a n", p=128))
            kv_sb = data.tile([128, 2, NK], f16, tag="kv_sb")
            nc.sync.dma_start(out=kv_sb, in_=kv_d.rearrange("(a p) n -> p a n", p=128))

            q_sb = acts.tile([128, 2, NQ_CORE], f16, tag="q_sb")
            k_sb = acts.tile([128, 2, NK], f16, tag="k_sb")
            vT_aug = acts.tile([128, 24, NH, D + 1], f16, tag="vT_aug")
            nc.vector.memset(vT_aug[:, :, :, D:D + 1], 1.0)

            # ---- projections (bias folded into the PSUM->SBUF eviction) ----
            with tc.tile_pool(name="proj_ps", bufs=4, space="PSUM") as proj_ps:
                # q = qw @ x + qb   (chunks of output channels x 512 cols)
                for mc in range(2):
                    for nb in range(2):
                        ps = proj_ps.tile([128, 512], f32, tag="proj")
                        for kc2 in range(2):
                            nc.tensor.matmul(
                                ps, lhsT=w_sb["qwT"][:, kc2, mc * 128:(mc + 1) * 128],
                                rhs=x_sb[:, kc2, nb * 512:(nb + 1) * 512],
                                start=(kc2 == 0), stop=(kc2 == 1))
                        nc.vector.tensor_scalar_add(
                            out=q_sb[:, mc, nb * 512:(nb + 1) * 512], in0=ps,
                            scalar1=qb_col[:, mc:mc + 1])
                # k = kw @ kv + kb
                for mc in range(2):
                    for nb in range(6):
                        ps = proj_ps.tile([128, 512], f32, tag="proj")
                        for kc2 in range(2):
                            nc.tensor.matmul(
                                ps, lhsT=w_sb["kwT"][:, kc2, mc * 128:(mc + 1) * 128],
                                rhs=kv_sb[:, kc2, nb * 512:(nb + 1) * 512],
                                start=(kc2 == 0), stop=(kc2 == 1))
                        nc.vector.tensor_scalar_add(
                            out=k_sb[:, mc, nb * 512:(nb + 1) * 512], in0=ps,
                            scalar1=kb_col[:, mc:mc + 1])
                # vT[n, c] = (kv^T @ vw^T)[n, c] + vb[c], written per-head with
                # a ones column appended (softmax denominator trick)
                for nn in range(24):
                    ps = proj_ps.tile([128, C], f32, tag="projv")
                    for kc2 in range(2):
                        nc.tensor.matmul(
                            ps, lhsT=kv_sb[:, kc2, nn * 128:(nn + 1) * 128],
                            rhs=w_sb["vwT"][:, kc2, :], start=(kc2 == 0),
                            stop=(kc2 == 1))
                    nc.vector.tensor_add(
                        vT_aug[:, nn, :, 0:D],
                        ps.rearrange("p (h e) -> p h e", h=NH),
                        vb_bc.rearrange("p (h e) -> p h e", h=NH))

            # ---- attention + o-proj + LN ----
            with tc.tile_pool(name="s_ps", bufs=3, space="PSUM") as s_pool, \
                 tc.tile_pool(name="o_ps", bufs=2, space="PSUM") as o_pool, \
                 tc.tile_pool(name="exps", bufs=3) as exp_pool, \
                 tc.tile_pool(name="tails", bufs=2) as tails, \
                 tc.tile_pool(name="norms", bufs=2) as norms, \
                 tc.tile_pool(name="fins", bufs=2) as fins:
                for qb in range(2):
                    # onrm[hg] accumulates the 4 normalized heads of chunk hg
                    onrm0 = norms.tile([128, 512], f16, tag="onrm0")
                    onrm1 = norms.tile([128, 512], f16, tag="onrm1")
                    onrm_tiles = [onrm0, onrm1]
                    for hp in range(4):          # head pairs
                        hg, sub = hp // 2, hp % 2
                        # attn@V col-packed: head A -> out partitions [0:33],
                        # head B -> [64:97] of ONE psum bank (col groups
                        # {0,1} and {2,3} run concurrently)
                        po = o_pool.tile([128, 512], f32, tag="opo")
                        for kc in range(24):
                            ps = s_pool.tile([128, 2, 512], f32, tag="S")
                            for j in range(2):
                                pof = 64 * sub + 32 * j
                                nc.tensor.matmul(
                                    ps[:, j, :],
                                    lhsT=k_sb[pof:pof + 32, hg, kc * 128:(kc + 1) * 128],
                                    rhs=q_sb[pof:pof + 32, hg, qb * 512:(qb + 1) * 512],
                                    start=True, stop=True, tile_position=(pof, 0))
                            slot = (qb * 4 + hp) * 24 + kc
                            if _use_dve_exp(slot):
                                es_i = exp_pool.tile([128, 2, 512], i16, tag="exp")
                                nc.vector.tensor_scalar(
                                    out=es_i, in0=ps, scalar1=_SCHR_A16,
                                    scalar2=_SCHR_B16, op0=Alu.mult, op1=Alu.add)
                                es = es_i.bitcast(f16)
                            else:
                                es = exp_pool.tile([128, 2, 512], f16, tag="exp")
                                nc.scalar.activation(es, ps, Act.Exp, scale=SCALE)
                            for j in range(2):
                                nc.tensor.matmul(
                                    po[64 * j:64 * j + D + 1, :],
                                    lhsT=vT_aug[:, kc, hp * 2 + j, :],
                                    rhs=es[:, j, :],
                                    start=(kc == 0), stop=(kc == 23),
                                    tile_position=(0, 64 * j))
                        # tail: numerators at po[0:32]/po[64:96], softmax
                        # denominators on single partition rows 32 / 96;
                        # iterative reciprocal is 8cyc/elem/lane, so spread
                        # the 1024 values over 32 partitions via DMA, recip,
                        # pack back
                        raw = tails.tile([128, 512], f32, tag="raw")
                        nc.vector.tensor_copy(raw, po)
                        dp = tails.tile([32, 32], f32, tag="dp")
                        nc.sync.dma_start(out=dp[0:16, :], in_=raw[D:D + 1, :])
                        nc.sync.dma_start(out=dp[16:32, :], in_=raw[64 + D:64 + D + 1, :])
                        rp = tails.tile([32, 32], f32, tag="rp")
                        nc.vector.reciprocal(rp, dp)
                        rec = tails.tile([1, 2, 512], f32, tag="rec")
                        nc.sync.dma_start(
                            out=rec.rearrange("p a q -> p (a q)"), in_=rp)
                        oin = tails.tile([128, 512], f32, tag="oin")
                        rbc = tails.tile([128, 512], f32, tag="rbc")
                        for j in range(2):
                            pof = 64 * sub + 32 * j
                            nc.sync.dma_start(out=oin[pof:pof + 32, :],
                                              in_=raw[64 * j:64 * j + D, :])
                            nc.sync.dma_start(out=rbc[pof:pof + 32, :],
                                              in_=bcast_sbuf_row(rec[0:1, j, :], 32))
                        nc.vector.tensor_mul(
                            onrm_tiles[hg][64 * sub:64 * sub + 64, :],
                            oin[64 * sub:64 * sub + 64, :],
                            rbc[64 * sub:64 * sub + 64, :])
                    # o-projection + residual + bias + LayerNorm per 128 queries
                    for qc2 in range(4):
                        qoff = qb * 512 + qc2 * 128
                        pso = s_pool.tile([128, C], f32, tag="S")
                        for hgc in range(2):
                            nc.tensor.matmul(
                                pso, lhsT=onrm_tiles[hgc][:, qc2 * 128:(qc2 + 1) * 128],
                                rhs=w_sb["owT"][:, hgc, :],
                                start=(hgc == 0), stop=False)
                        for cc in range(2):
                            nc.tensor.matmul(
                                pso, lhsT=x_sb[:, cc, qoff:qoff + 128],
                                rhs=w_sb["ident"][:, cc, :], start=False, stop=False)
                        nc.tensor.matmul(pso, lhsT=ones_row[0:1, 0:128], rhs=ob_row[:],
                                         start=False, stop=True)
                        stats = fins.tile([128, 6], f32, tag="stats")
                        nc.vector.bn_stats(stats, pso)
                        mv = fins.tile([128, 2], f32, tag="mv")
                        nc.vector.bn_aggr(mv, stats)
                        # rstd = exp(-0.5*ln(var+eps)): stays in the same ACT
                        # table set as the softmax exp (no table reload)
                        lnv = fins.tile([128, 1], f32, tag="lnv")
                        nc.scalar.activation(lnv, mv[:, 1:2], Act.Ln, bias=eps_col[:, 0:1])
                        rstd = fins.tile([128, 1], f32, tag="rstd")
                        nc.scalar.activation(rstd, lnv, Act.Exp, scale=-0.5)
                        t1 = fins.tile([128, C], f32, tag="t1")
                        nc.vector.tensor_scalar(
                            out=t1, in0=pso, scalar1=mv[:, 0:1], scalar2=rstd,
                            op0=Alu.subtract, op1=Alu.mult)
                        t2 = fins.tile([128, C], f32, tag="t2")
                        nc.vector.tensor_mul(t2, t1, lnw_bc)
                        t3 = fins.tile([128, C], f32, tag="t3")
                        nc.vector.tensor_add(t3, t2, lnb_bc)
                        nc.sync.dma_start(out=y_d[qoff:qoff + 128, :], in_=t3)
    return nc


_CACHE = {}


def _get_program():
    if "nc" not in _CACHE:
        _apply_walrus_wait_patch()
        _CACHE["nc"] = build_program()
    return _CACHE["nc"]


def _make_in_maps(inputs):
    s3 = np.asarray(inputs["s3"], dtype=np.float32)
    s4 = np.asarray(inputs["s4"], dtype=np.float32)
    s5 = np.asarray(inputs["s5"], dtype=np.float32)
    wts = {}
    for nm in ("qw", "kw", "vw", "ow"):
        wts[nm + "T"] = np.ascontiguousarray(
            np.asarray(inputs[nm], dtype=np.float32).T.astype(np.float16))
    ident = np.eye(C, dtype=np.float16)
    qb_t = np.ascontiguousarray(
        np.asarray(inputs["qb"], dtype=np.float32).reshape(2, 128).T)
    kb_t = np.ascontiguousarray(
        np.asarray(inputs["kb"], dtype=np.float32).reshape(2, 128).T)
    vb_r = np.asarray(inputs["vb"], dtype=np.float32).reshape(1, C)
    ob_r = np.asarray(inputs["ob"], dtype=np.float32).reshape(1, C).astype(np.float16)
    lnw_r = np.asarray(inputs["ln_w"], dtype=np.float32).reshape(1, C)
    lnb_r = np.asarray(inputs["ln_b"], dtype=np.float32).reshape(1, C)
    in_maps = []
    for core in range(N_CORES):
        b, qc = core // 4, core % 4
        x = np.ascontiguousarray(
            s3[b].reshape(C, -1)[:, qc * NQ_CORE:(qc + 1) * NQ_CORE]
        ).astype(np.float16)
        kv = np.concatenate(
            [s4[b].reshape(C, -1), s5[b].reshape(C, -1)], axis=1).astype(np.float16)
        in_maps.append({
            "x": x, "kv": kv,
            "qwT": wts["qwT"], "kwT": wts["kwT"], "vwT": wts["vwT"],
            "owT": wts["owT"], "ident": ident,
            "qb2": qb_t, "kb2": kb_t, "vb2": vb_r,
            "ob2": ob_r, "lnw2": lnw_r, "lnb2": lnb_r,
        })
    return in_maps


def _assemble(results, like):
    B, _, H, W = 2, C, 64, 64
    out = np.empty((B, C, H * W), dtype=np.float32)
    for core in range(N_CORES):
        b, qc = core // 4, core % 4
        out[b, :, qc * NQ_CORE:(qc + 1) * NQ_CORE] = results[core]["y"].T
    return out.reshape(B, C, H, W)


def kernel(**inputs):
    from concourse import bass2jax
    nc = _get_program()
    in_maps = _make_in_maps(inputs)
    results = bass2jax.run_bass_via_pjrt(nc, in_maps, n_cores=N_CORES)
    return _assemble(results, inputs["s3"])


# revision 6
# speedup vs baseline: 2.5522x; 1.0359x over previous
"""Content-guided attention kernel for Trainium2, 8 NeuronCores SPMD.

Sharding: 8 cores = (batch b in {0,1}) x (query-chunk qc in {0..3}).
Each core computes 1024 query positions of batch b end-to-end:
q/k/vT projections, 8-head attention over all 3072 keys, o-projection,
residual and LayerNorm.  No collectives needed; host splits/concats.

Per-core layout highlights:
 - all matmul operands in fp16 (1 cycle/row on the PE vs 4 for fp32;
   PSUM accumulation stays fp32)
 - scores computed transposed S^T[kpos, qpos] so softmax sum folds into the
   attn@V matmul via a ones-column appended to V^T (no partition reductions)
 - head_dim=32 scores matmuls are packed 4-at-a-time into the PE's 32-row
   groups via tile_position (4x concurrency at K=32)
 - attn@V matmuls col-packed 2-at-a-time (heads at output partitions 0 and
   64 of one PSUM bank) so the M=33 lane waste halves
 - normalized head outputs stay in the attn@V partition layout; the o-proj
   weight matrix is row-permuted and zero-padded host-side to match, which
   removes the per-head SBUF shuffle DMAs of the numerators
 - o-projection of query block qb is emitted AFTER the first head-pair of
   block qb+1 so the softmax-normalize tail latency hides behind PE work
 - q/k biases folded into the PSUM->SBUF eviction tensor_scalar; v bias via
   tensor_tensor add against a partition-broadcast tile
 - exp split between ScalarE (table exp, fp16 out) and VectorE (Schraudolph
   bit-trick exp emitted as int16 fp16-bitpattern)
 - LayerNorm rstd computed as exp(-0.5*ln(var+eps)) to stay inside the
   single natural_log_exp ACT table set (no table switch thrash)
"""

import numpy as np

C = 256
NH = 8
D = 32
NQ_CORE = 1024
NK = 3072
N_CORES = 8
SCALE = float(D) ** -0.5

# Schraudolph exp constants for an fp16 bit-pattern target:
# bits16 = round(s * SCALE * 1024/ln2 + (15*1024 - 44.56))
_SCHR_A16 = float(np.float32(SCALE * 1024.0 / np.log(2.0)))
_SCHR_B16 = float(np.float32(15.0 * 1024.0 - 44.56))

# exp slots: 3 of every 5 on ScalarE (table exp), 2 of 5 on VectorE
def _use_dve_exp(slot: int) -> bool:
    return slot % 5 >= 3


def _apply_walrus_wait_patch():
    """This walrus build accepts only ONE sync-wait per instruction; split
    extra waits onto single-wait NoOps inserted before the instruction
    (same engine, same block => per-engine program order preserved)."""
    import orjson
    import concourse.bass_utils as bass_utils
    import concourse.bass2jax as bass2jax

    if getattr(bass_utils, "_ant_wait_split_patch", False):
        return
    bass_utils._ant_wait_split_patch = True
    counter = [0]

    def _split_waits(bir_bytes: bytes) -> bytes:
        d = orjson.loads(bir_bytes)
        changed = False

        def process_blocks(blocks):
            nonlocal changed
            for b in blocks:
                insts = b.get("instructions")
                if insts:
                    new = []
                    for ins in insts:
                        si = ins.get("sync_info")
                        waits = si.get("on_wait") if si else None
                        if waits and len(waits) > 1:
                            changed = True
                            for w in waits[:-1]:
                                counter[0] += 1
                                new.append({
                                    "debug": ins.get("debug", 0),
                                    "engine": ins["engine"],
                                    "ins": [],
                                    "outs": [],
                                    "name": f"antwsplit-{counter[0]}",
                                    "opcode": "NoOp",
                                    "sync_info": {"on_wait": [w], "on_update": []},
                                })
                            si["on_wait"] = [waits[-1]]
                        new.append(ins)
                    b["instructions"] = new
                if b.get("blocks"):
                    process_blocks(b["blocks"])

        for f in d.get("functions", []):
            process_blocks(f.get("blocks", []))
        return orjson.dumps(d) if changed else bir_bytes

    orig = bass_utils.compile_bir_kernel

    def compile_bir_kernel(bir, tmpdir, neff_name="file.neff", **kw):
        if isinstance(bir, (bytes, bytearray)):
            bir = _split_waits(bytes(bir))
        elif isinstance(bir, str):
            bir = _split_waits(bir.encode()).decode()
        return orig(bir, tmpdir, neff_name=neff_name, **kw)

    bass_utils.compile_bir_kernel = compile_bir_kernel
    bass2jax.compile_bir_kernel = compile_bir_kernel


def build_program():
    import concourse.bass as bass
    import concourse.tile as tile
    from concourse import mybir

    f32 = mybir.dt.float32
    f16 = mybir.dt.float16
    i16 = mybir.dt.int16
    Alu = mybir.AluOpType
    Act = mybir.ActivationFunctionType

    nc = bass.Bass()

    x_d = nc.dram_tensor("x", [C, NQ_CORE], f16, kind="ExternalInput")
    kv_d = nc.dram_tensor("kv", [C, NK], f16, kind="ExternalInput")
    qwT_d = nc.dram_tensor("qwT", [C, C], f16, kind="ExternalInput")
    kwT_d = nc.dram_tensor("kwT", [C, C], f16, kind="ExternalInput")
    vwT_d = nc.dram_tensor("vwT", [C, C], f16, kind="ExternalInput")
    owA_d = nc.dram_tensor("owA", [4 * 128, C], f16, kind="ExternalInput")
    ident_d = nc.dram_tensor("ident", [C, C], f16, kind="ExternalInput")
    qb_d = nc.dram_tensor("qb2", [128, 2], f32, kind="ExternalInput")
    kb_d = nc.dram_tensor("kb2", [128, 2], f32, kind="ExternalInput")
    vb_d = nc.dram_tensor("vb2", [1, C], f32, kind="ExternalInput")
    ob_d = nc.dram_tensor("ob2", [1, C], f16, kind="ExternalInput")
    lnw_d = nc.dram_tensor("lnw2", [1, C], f32, kind="ExternalInput")
    lnb_d = nc.dram_tensor("lnb2", [1, C], f32, kind="ExternalInput")
    y_d = nc.dram_tensor("y", [NQ_CORE, C], f32, kind="ExternalOutput")

    def bcast_part(ap, n):
        # partition-stride-0 view: replicate one partition row across n
        # (DRAM sources only; SBUF partition dims need nonzero step)
        return bass.AP(tensor=ap.tensor, offset=ap.offset,
                       ap=[[0, n]] + [list(a) for a in ap.ap[1:]])

    def bcast_sbuf_row(ap, n):
        # SBUF [1, F] row -> [n, F] DMA source: keep the 1-partition dim,
        # replicate via a step-0 free dim (legal for DMA reads)
        return bass.AP(tensor=ap.tensor, offset=ap.offset,
                       ap=[list(ap.ap[0]), [0, n]] + [list(a) for a in ap.ap[1:]])

    from contextlib import ExitStack
    with tile.TileContext(nc) as tc, ExitStack() as ctx:
            consts = ctx.enter_context(tc.tile_pool(name="consts", bufs=1))
            data = ctx.enter_context(tc.tile_pool(name="data", bufs=1))
            acts = ctx.enter_context(tc.tile_pool(name="acts", bufs=1))
            # ---- constants ----
            w_sb = {}
            for nm, dt_ in (("qwT", qwT_d), ("kwT", kwT_d), ("vwT", vwT_d),
                            ("ident", ident_d)):
                t = consts.tile([128, 2, C], f16, tag=f"w_{nm}")
                nc.sync.dma_start(out=t, in_=dt_.rearrange("(a p) c -> p a c", p=128))
                w_sb[nm] = t
            owA_sb = consts.tile([128, 4, C], f16, tag="w_owA")
            nc.sync.dma_start(out=owA_sb,
                              in_=owA_d.rearrange("(a p) c -> p a c", p=128))
            qb_col = consts.tile([128, 2], f32, tag="qb_col")
            kb_col = consts.tile([128, 2], f32, tag="kb_col")
            nc.sync.dma_start(out=qb_col, in_=qb_d[:])
            nc.sync.dma_start(out=kb_col, in_=kb_d[:])
            ob_row = consts.tile([1, C], f16, tag="ob_row")
            nc.sync.dma_start(out=ob_row, in_=ob_d[:])
            vb_bc = consts.tile([128, C], f32, tag="vb_bc")
            nc.sync.dma_start(out=vb_bc, in_=bcast_part(vb_d[:], 128))
            lnw_bc = consts.tile([128, C], f32, tag="lnw_bc")
            lnb_bc = consts.tile([128, C], f32, tag="lnb_bc")
            nc.sync.dma_start(out=lnw_bc, in_=bcast_part(lnw_d[:], 128))
            nc.sync.dma_start(out=lnb_bc, in_=bcast_part(lnb_d[:], 128))
            ones_row = consts.tile([1, 512], f16, tag="ones_row")
            nc.vector.memset(ones_row, 1.0)
            eps_col = consts.tile([128, 1], f32, tag="eps_col")
            nc.vector.memset(eps_col, 1e-5)

            # ---- input activations (chunked DMAs so compute starts early) ----
            x_sb = data.tile([128, 2, NQ_CORE], f16, tag="x_sb")
            x_r = x_d.rearrange("(a p) n -> p a n", p=128)
            for nb in range(2):
                nc.sync.dma_start(out=x_sb[:, :, nb * 512:(nb + 1) * 512],
                                  in_=x_r[:, :, nb * 512:(nb + 1) * 512])
            kv_sb = data.tile([128, 2, NK], f16, tag="kv_sb")
            kv_r = kv_d.rearrange("(a p) n -> p a n", p=128)
            for nb in range(6):
                nc.sync.dma_start(out=kv_sb[:, :, nb * 512:(nb + 1) * 512],
                                  in_=kv_r[:, :, nb * 512:(nb + 1) * 512])

            q_sb = acts.tile([128, 2, NQ_CORE], f16, tag="q_sb")
            k_sb = acts.tile([128, 2, NK], f16, tag="k_sb")
            vT_aug = acts.tile([128, 24, NH, D + 1], f16, tag="vT_aug")
            nc.vector.memset(vT_aug[:, :, :, D:D + 1], 1.0)
            # onrm[qb][hp]: normalized attn@V numerators in po layout
            # (head A rows 0:32, head B rows 64:96).  Rows 32:64 / 96:128 are
            # dead lanes multiplied by zero-padded owA rows in the o-proj;
            # memset once so stale SBUF NaNs can't propagate through 0*x.
            onrm = [[acts.tile([128, 512], f16, tag=f"onrm_{qb}_{hp}",
                                name=f"onrm_{qb}_{hp}")
                     for hp in range(4)] for qb in range(2)]
            for qb in range(2):
                for hp in range(4):
                    nc.vector.memset(onrm[qb][hp][32:64, :], 0.0)
                    nc.vector.memset(onrm[qb][hp][96:128, :], 0.0)

            # ---- projections (bias folded into the PSUM->SBUF eviction) ----
            with tc.tile_pool(name="proj_ps", bufs=4, space="PSUM") as proj_ps:
                # q = qw @ x + qb   (chunks of output channels x 512 cols)
                for mc in range(2):
                    for nb in range(2):
                        ps = proj_ps.tile([128, 512], f32, tag="proj")
                        for kc2 in range(2):
                            nc.tensor.matmul(
                                ps, lhsT=w_sb["qwT"][:, kc2, mc * 128:(mc + 1) * 128],
                                rhs=x_sb[:, kc2, nb * 512:(nb + 1) * 512],
                                start=(kc2 == 0), stop=(kc2 == 1))
                        nc.vector.tensor_scalar_add(
                            out=q_sb[:, mc, nb * 512:(nb + 1) * 512], in0=ps,
                            scalar1=qb_col[:, mc:mc + 1])
                # k = kw @ kv + kb ; vT = kv^T @ vw^T + vb, interleaved per
                # kv chunk so compute starts as soon as each chunk lands
                for nb in range(6):
                    for mc in range(2):
                        ps = proj_ps.tile([128, 512], f32, tag="proj")
                        for kc2 in range(2):
                            nc.tensor.matmul(
                                ps, lhsT=w_sb["kwT"][:, kc2, mc * 128:(mc + 1) * 128],
                                rhs=kv_sb[:, kc2, nb * 512:(nb + 1) * 512],
                                start=(kc2 == 0), stop=(kc2 == 1))
                        nc.vector.tensor_scalar_add(
                            out=k_sb[:, mc, nb * 512:(nb + 1) * 512], in0=ps,
                            scalar1=kb_col[:, mc:mc + 1])
                    for nn in range(4 * nb, 4 * nb + 4):
                        ps = proj_ps.tile([128, C], f32, tag="projv")
                        for kc2 in range(2):
                            nc.tensor.matmul(
                                ps, lhsT=kv_sb[:, kc2, nn * 128:(nn + 1) * 128],
                                rhs=w_sb["vwT"][:, kc2, :], start=(kc2 == 0),
                                stop=(kc2 == 1))
                        nc.vector.tensor_add(
                            vT_aug[:, nn, :, 0:D],
                            ps.rearrange("p (h e) -> p h e", h=NH),
                            vb_bc.rearrange("p (h e) -> p h e", h=NH))

            # ---- attention + o-proj + LN ----
            with tc.tile_pool(name="s_ps", bufs=3, space="PSUM") as s_pool, \
                 tc.tile_pool(name="o_ps", bufs=2, space="PSUM") as o_pool, \
                 tc.tile_pool(name="exps", bufs=3) as exp_pool, \
                 tc.tile_pool(name="tails", bufs=2) as tails, \
                 tc.tile_pool(name="fins", bufs=2) as fins:

                def emit_head_pair(qb, hp):
                    hg, sub = hp // 2, hp % 2
                    # attn@V col-packed: head A -> out partitions [0:33],
                    # head B -> [64:97] of ONE psum bank (col groups
                    # {0,1} and {2,3} run concurrently)
                    po = o_pool.tile([128, 512], f32, tag="opo")
                    for kc in range(24):
                        ps = s_pool.tile([128, 2, 512], f32, tag="S")
                        for j in range(2):
                            pof = 64 * sub + 32 * j
                            nc.tensor.matmul(
                                ps[:, j, :],
                                lhsT=k_sb[pof:pof + 32, hg, kc * 128:(kc + 1) * 128],
                                rhs=q_sb[pof:pof + 32, hg, qb * 512:(qb + 1) * 512],
                                start=True, stop=True, tile_position=(pof, 0))
                        slot = (qb * 4 + hp) * 24 + kc
                        if _use_dve_exp(slot):
                            es_i = exp_pool.tile([128, 2, 512], i16, tag="exp")
                            nc.vector.tensor_scalar(
                                out=es_i, in0=ps, scalar1=_SCHR_A16,
                                scalar2=_SCHR_B16, op0=Alu.mult, op1=Alu.add)
                            es = es_i.bitcast(f16)
                        else:
                            es = exp_pool.tile([128, 2, 512], f16, tag="exp")
                            nc.scalar.activation(es, ps, Act.Exp, scale=SCALE)
                        for j in range(2):
                            nc.tensor.matmul(
                                po[64 * j:64 * j + D + 1, :],
                                lhsT=vT_aug[:, kc, hp * 2 + j, :],
                                rhs=es[:, j, :],
                                start=(kc == 0), stop=(kc == 23),
                                tile_position=(0, 64 * j))
                    # tail: numerators stay put in po layout; only the two
                    # denominator rows (32 / 96) leave PSUM.  Reciprocal is
                    # 8cyc/elem/lane, so spread the 1024 values over 32
                    # partitions via DMA, recip, broadcast back per head.
                    raw = tails.tile([128, 512], f32, tag="raw")
                    nc.vector.tensor_copy(raw, po)
                    dp = tails.tile([32, 32], f32, tag="dp")
                    nc.sync.dma_start(out=dp[0:16, :], in_=raw[D:D + 1, :])
                    nc.sync.dma_start(out=dp[16:32, :], in_=raw[64 + D:64 + D + 1, :])
                    rp = tails.tile([32, 32], f32, tag="rp")
                    nc.vector.reciprocal(rp, dp)
                    rec = tails.tile([1, 2, 512], f32, tag="rec")
                    nc.sync.dma_start(
                        out=rec.rearrange("p a q -> p (a q)"), in_=rp)
                    rbc = tails.tile([128, 512], f32, tag="rbc")
                    for j in range(2):
                        nc.sync.dma_start(out=rbc[64 * j:64 * j + 32, :],
                                          in_=bcast_sbuf_row(rec[0:1, j, :], 32))
                    for j in range(2):
                        nc.vector.tensor_mul(
                            onrm[qb][hp][64 * j:64 * j + 32, :],
                            po[64 * j:64 * j + 32, :],
                            rbc[64 * j:64 * j + 32, :])

                def emit_oproj(qb):
                    # o-projection + residual + bias + LayerNorm per 128 queries
                    for qc2 in range(4):
                        qoff = qb * 512 + qc2 * 128
                        pso = s_pool.tile([128, C], f32, tag="S")
                        for hp in range(4):
                            nc.tensor.matmul(
                                pso, lhsT=onrm[qb][hp][:, qc2 * 128:(qc2 + 1) * 128],
                                rhs=owA_sb[:, hp, :],
                                start=(hp == 0), stop=False)
                        for cc in range(2):
                            nc.tensor.matmul(
                                pso, lhsT=x_sb[:, cc, qoff:qoff + 128],
                                rhs=w_sb["ident"][:, cc, :], start=False, stop=False)
                        nc.tensor.matmul(pso, lhsT=ones_row[0:1, 0:128], rhs=ob_row[:],
                                         start=False, stop=True)
                        stats = fins.tile([128, 6], f32, tag="stats")
                        nc.vector.bn_stats(stats, pso)
                        mv = fins.tile([128, 2], f32, tag="mv")
                        nc.vector.bn_aggr(mv, stats)
                        # rstd = exp(-0.5*ln(var+eps)): stays in the same ACT
                        # table set as the softmax exp (no table reload)
                        lnv = fins.tile([128, 1], f32, tag="lnv")
                        nc.scalar.activation(lnv, mv[:, 1:2], Act.Ln, bias=eps_col[:, 0:1])
                        rstd = fins.tile([128, 1], f32, tag="rstd")
                        nc.scalar.activation(rstd, lnv, Act.Exp, scale=-0.5)
                        t1 = fins.tile([128, C], f32, tag="t1")
                        nc.vector.tensor_scalar(
                            out=t1, in0=pso, scalar1=mv[:, 0:1], scalar2=rstd,
                            op0=Alu.subtract, op1=Alu.mult)
                        t2 = fins.tile([128, C], f32, tag="t2")
                        nc.vector.tensor_mul(t2, t1, lnw_bc)
                        t3 = fins.tile([128, C], f32, tag="t3")
                        nc.vector.tensor_add(t3, t2, lnb_bc)
                        nc.sync.dma_start(out=y_d[qoff:qoff + 128, :], in_=t3)

                # software pipeline: o-proj of block qb emitted after the
                # first head-pair of block qb+1 so the softmax tail latency
                # hides behind that head-pair's PE work
                for qb in range(2):
                    for hp in range(4):
                        emit_head_pair(qb, hp)
                        if qb == 1 and hp == 0:
                            emit_oproj(0)
                emit_oproj(1)
    return nc


_CACHE = {}


def _get_program():
    if "nc" not in _CACHE:
        _apply_walrus_wait_patch()
        _CACHE["nc"] = build_program()
    return _CACHE["nc"]


def _make_in_maps(inputs):
    s3 = np.asarray(inputs["s3"], dtype=np.float32)
    s4 = np.asarray(inputs["s4"], dtype=np.float32)
    s5 = np.asarray(inputs["s5"], dtype=np.float32)
    wts = {}
    for nm in ("qw", "kw", "vw"):
        wts[nm + "T"] = np.ascontiguousarray(
            np.asarray(inputs[nm], dtype=np.float32).T.astype(np.float16))
    # o-proj weights permuted + zero-padded to the attn@V PSUM layout:
    # chunk hp rows = [head 2hp (32) | zeros (32) | head 2hp+1 (32) | zeros]
    owT = np.asarray(inputs["ow"], dtype=np.float32).T.astype(np.float16)
    owA = np.zeros((4, 128, C), dtype=np.float16)
    for hp in range(4):
        owA[hp, 0:32] = owT[(2 * hp) * 32:(2 * hp) * 32 + 32]
        owA[hp, 64:96] = owT[(2 * hp + 1) * 32:(2 * hp + 1) * 32 + 32]
    owA = np.ascontiguousarray(owA.reshape(4 * 128, C))
    ident = np.eye(C, dtype=np.float16)
    qb_t = np.ascontiguousarray(
        np.asarray(inputs["qb"], dtype=np.float32).reshape(2, 128).T)
    kb_t = np.ascontiguousarray(
        np.asarray(inputs["kb"], dtype=np.float32).reshape(2, 128).T)
    vb_r = np.asarray(inputs["vb"], dtype=np.float32).reshape(1, C)
    ob_r = np.asarray(inputs["ob"], dtype=np.float32).reshape(1, C).astype(np.float16)
    lnw_r = np.asarray(inputs["ln_w"], dtype=np.float32).reshape(1, C)
    lnb_r = np.asarray(inputs["ln_b"], dtype=np.float32).reshape(1, C)
    in_maps = []
    for core in range(N_CORES):
        b, qc = core // 4, core % 4
        x = np.ascontiguousarray(
            s3[b].reshape(C, -1)[:, qc * NQ_CORE:(qc + 1) * NQ_CORE]
        ).astype(np.float16)
        kv = np.concatenate(
            [s4[b].reshape(C, -1), s5[b].reshape(C, -1)], axis=1).astype(np.float16)
        in_maps.append({
            "x": x, "kv": kv,
            "qwT": wts["qwT"], "kwT": wts["kwT"], "vwT": wts["vwT"],
            "owA": owA, "ident": ident,
            "qb2": qb_t, "kb2": kb_t, "vb2": vb_r,
            "ob2": ob_r, "lnw2": lnw_r, "lnb2": lnb_r,
        })
    return in_maps


def _assemble(results, like):
    B, _, H, W = 2, C, 64, 64
    out = np.empty((B, C, H * W), dtype=np.float32)
    for core in range(N_CORES):
        b, qc = core // 4, core % 4
        out[b, :, qc * NQ_CORE:(qc + 1) * NQ_CORE] = results[core]["y"].T
    return out.reshape(B, C, H, W)


def kernel(**inputs):
    from concourse import bass2jax
    nc = _get_program()
    in_maps = _make_in_maps(inputs)
    results = bass2jax.run_bass_via_pjrt(nc, in_maps, n_cores=N_CORES)
    return _assemble(results, inputs["s3"])


# revision 9
# speedup vs baseline: 2.6181x; 1.0258x over previous
"""Content-guided attention kernel for Trainium2, 8 NeuronCores SPMD.

Sharding: 8 cores = (batch b in {0,1}) x (query-chunk qc in {0..3}).
Each core computes 1024 query positions of batch b end-to-end:
q/k/vT projections, 8-head attention over all 3072 keys, o-projection,
residual and LayerNorm.  No collectives needed; host splits/concats.

Per-core layout highlights:
 - all matmul operands in fp16 (1 cycle/row on the PE vs 4 for fp32;
   PSUM accumulation stays fp32)
 - scores computed transposed S^T[kpos, qpos] so softmax sum folds into the
   attn@V matmul via a ones-column appended to V^T (no partition reductions)
 - head_dim=32 scores matmuls are packed 4-at-a-time into the PE's 32-row
   groups via tile_position (4x concurrency at K=32)
 - attn@V matmuls col-packed 2-at-a-time (heads at output partitions 0 and
   64 of one PSUM bank) so the M=33 lane waste halves
 - attn@V of score-chunk u issues 2 chunks after its scores, so the exp
   result is always ready when the PE reaches it: the PE stream is
   gap-free, which keeps the HAM clock-gate at full rate (2.4 GHz) instead
   of oscillating back to the cold 1.2 GHz state
 - normalized head outputs stay in the attn@V partition layout; the o-proj
   weight matrix is row-permuted and zero-padded host-side to match, which
   removes the per-head SBUF shuffle DMAs of the numerators
 - o-projection of query block 0 is emitted a few score-chunks into block 1
   so the softmax-normalize tail latency hides behind PE work
 - inputs arrive via few, large DMAs split across both HWDGE queues (SP +
   Activation) because each dma_start costs ~0.65us of trigger time
 - q/k biases folded into the PSUM->SBUF eviction tensor_scalar; v bias via
   tensor_tensor add against a partition-broadcast tile
 - exp split between ScalarE (table exp, fp16 out) and VectorE (Schraudolph
   bit-trick exp emitted as int16 fp16-bitpattern)
 - LayerNorm rstd computed as exp(-0.5*ln(var+eps)) to stay inside the
   single natural_log_exp ACT table set; LN scale/shift run on GpSimd
"""

import numpy as np

C = 256
NH = 8
D = 32
NQ_CORE = 1024
NK = 3072
N_CORES = 8
SCALE = float(D) ** -0.5

# Schraudolph exp constants for an fp16 bit-pattern target:
# bits16 = round(s * SCALE * 1024/ln2 + (15*1024 - 44.56))
_SCHR_A16 = float(np.float32(SCALE * 1024.0 / np.log(2.0)))
_SCHR_B16 = float(np.float32(15.0 * 1024.0 - 44.56))

# exp slots: 3 of every 5 on ScalarE (table exp), 2 of 5 on VectorE
def _use_dve_exp(slot: int) -> bool:
    return slot % 5 >= 3


def _apply_walrus_wait_patch():
    """This walrus build accepts only ONE sync-wait per instruction; split
    extra waits onto single-wait NoOps inserted before the instruction
    (same engine, same block => per-engine program order preserved)."""
    import orjson
    import concourse.bass_utils as bass_utils
    import concourse.bass2jax as bass2jax

    if getattr(bass_utils, "_ant_wait_split_patch", False):
        return
    bass_utils._ant_wait_split_patch = True
    counter = [0]

    def _split_waits(bir_bytes: bytes) -> bytes:
        d = orjson.loads(bir_bytes)
        changed = False

        def process_blocks(blocks):
            nonlocal changed
            for b in blocks:
                insts = b.get("instructions")
                if insts:
                    new = []
                    for ins in insts:
                        si = ins.get("sync_info")
                        waits = si.get("on_wait") if si else None
                        if waits and len(waits) > 1:
                            changed = True
                            for w in waits[:-1]:
                                counter[0] += 1
                                new.append({
                                    "debug": ins.get("debug", 0),
                                    "engine": ins["engine"],
                                    "ins": [],
                                    "outs": [],
                                    "name": f"antwsplit-{counter[0]}",
                                    "opcode": "NoOp",
                                    "sync_info": {"on_wait": [w], "on_update": []},
                                })
                            si["on_wait"] = [waits[-1]]
                        new.append(ins)
                    b["instructions"] = new
                if b.get("blocks"):
                    process_blocks(b["blocks"])

        for f in d.get("functions", []):
            process_blocks(f.get("blocks", []))
        return orjson.dumps(d) if changed else bir_bytes

    orig = bass_utils.compile_bir_kernel

    def compile_bir_kernel(bir, tmpdir, neff_name="file.neff", **kw):
        if isinstance(bir, (bytes, bytearray)):
            bir = _split_waits(bytes(bir))
        elif isinstance(bir, str):
            bir = _split_waits(bir.encode()).decode()
        return orig(bir, tmpdir, neff_name=neff_name, **kw)

    bass_utils.compile_bir_kernel = compile_bir_kernel
    bass2jax.compile_bir_kernel = compile_bir_kernel


def build_program():
    import concourse.bass as bass
    import concourse.tile as tile
    from concourse import mybir

    f32 = mybir.dt.float32
    f16 = mybir.dt.float16
    i16 = mybir.dt.int16
    Alu = mybir.AluOpType
    Act = mybir.ActivationFunctionType

    nc = bass.Bass()

    x_d = nc.dram_tensor("x", [C, NQ_CORE], f16, kind="ExternalInput")
    kv_d = nc.dram_tensor("kv", [C, NK], f16, kind="ExternalInput")
    # all fp16 weights stacked: qwT | kwT | vwT | ident | owA (4x128 rows)
    wall_d = nc.dram_tensor("wall", [12 * 128, C], f16, kind="ExternalInput")
    qkb_d = nc.dram_tensor("qkb", [128, 4], f32, kind="ExternalInput")
    rows3_d = nc.dram_tensor("rows3", [1, 3 * C], f32, kind="ExternalInput")
    ob_d = nc.dram_tensor("ob2", [1, C], f16, kind="ExternalInput")
    y_d = nc.dram_tensor("y", [NQ_CORE, C], f32, kind="ExternalOutput")

    def bcast_part(ap, n):
        # partition-stride-0 view: replicate one partition row across n
        # (DRAM sources only; SBUF partition dims need nonzero step)
        return bass.AP(tensor=ap.tensor, offset=ap.offset,
                       ap=[[0, n]] + [list(a) for a in ap.ap[1:]])

    def bcast_sbuf_row(ap, n):
        # SBUF [1, F] row -> [n, F] DMA source: keep the 1-partition dim,
        # replicate via a step-0 free dim (legal for DMA reads)
        return bass.AP(tensor=ap.tensor, offset=ap.offset,
                       ap=[list(ap.ap[0]), [0, n]] + [list(a) for a in ap.ap[1:]])

    from contextlib import ExitStack
    with tile.TileContext(nc) as tc, ExitStack() as ctx:
            consts = ctx.enter_context(tc.tile_pool(name="consts", bufs=1))
            data = ctx.enter_context(tc.tile_pool(name="data", bufs=1))
            acts = ctx.enter_context(tc.tile_pool(name="acts", bufs=1))
            # ---- inputs: few big DMAs, split across the two HWDGE queues.
            # sync queue: weights + x (unblocks q-proj first);
            # scalar queue: kv chunks + small consts.
            w_all = consts.tile([128, 12, C], f16, tag="w_all")
            nc.sync.dma_start(out=w_all,
                              in_=wall_d.rearrange("(a p) c -> p a c", p=128))
            W_Q, W_K, W_V, W_ID, W_OA = 0, 2, 4, 6, 8

            x_sb = data.tile([128, 2, NQ_CORE], f16, tag="x_sb")
            x_r = x_d.rearrange("(a p) n -> p a n", p=128)
            for nb in range(2):
                nc.sync.dma_start(out=x_sb[:, :, nb * 512:(nb + 1) * 512],
                                  in_=x_r[:, :, nb * 512:(nb + 1) * 512])
            kv_sb = data.tile([128, 2, NK], f16, tag="kv_sb")
            kv_r = kv_d.rearrange("(a p) n -> p a n", p=128)
            for nb in range(6):
                nc.scalar.dma_start(out=kv_sb[:, :, nb * 512:(nb + 1) * 512],
                                    in_=kv_r[:, :, nb * 512:(nb + 1) * 512])
            qkb_col = consts.tile([128, 4], f32, tag="qkb_col")
            nc.scalar.dma_start(out=qkb_col, in_=qkb_d[:])
            rows3_bc = consts.tile([128, 3 * C], f32, tag="rows3_bc")
            nc.scalar.dma_start(out=rows3_bc, in_=bcast_part(rows3_d[:], 128))
            vb_bc = rows3_bc[:, 0:C]
            lnw_bc = rows3_bc[:, C:2 * C]
            lnb_bc = rows3_bc[:, 2 * C:3 * C]
            ob_row = consts.tile([1, C], f16, tag="ob_row")
            nc.scalar.dma_start(out=ob_row, in_=ob_d[:])
            ones_row = consts.tile([1, 512], f16, tag="ones_row")
            nc.vector.memset(ones_row, 1.0)
            eps_col = consts.tile([128, 1], f32, tag="eps_col")
            nc.vector.memset(eps_col, 1e-5)

            q_sb = acts.tile([128, 2, NQ_CORE], f16, tag="q_sb")
            k_sb = acts.tile([128, 2, NK], f16, tag="k_sb")
            vT_aug = acts.tile([128, 24, NH, D + 1], f16, tag="vT_aug")
            nc.vector.memset(vT_aug[:, :, :, D:D + 1], 1.0)
            # onrm[qb][hp]: normalized attn@V numerators in po layout
            # (head A rows 0:32, head B rows 64:96).  Rows 32:64 / 96:128 are
            # dead lanes multiplied by zero-padded owA rows in the o-proj;
            # memset once so stale SBUF NaNs can't propagate through 0*x.
            onrm = [[acts.tile([128, 512], f16, tag=f"onrm_{qb}_{hp}",
                                name=f"onrm_{qb}_{hp}")
                     for hp in range(4)] for qb in range(2)]
            for qb in range(2):
                for hp in range(4):
                    nc.vector.memset(onrm[qb][hp][32:64, :], 0.0)
                    nc.vector.memset(onrm[qb][hp][96:128, :], 0.0)

            # ---- projections (bias folded into the PSUM->SBUF eviction) ----
            with tc.tile_pool(name="proj_ps", bufs=4, space="PSUM") as proj_ps:
                # q = qw @ x + qb   (chunks of output channels x 512 cols)
                for mc in range(2):
                    for nb in range(2):
                        ps = proj_ps.tile([128, 512], f32, tag="proj")
                        for kc2 in range(2):
                            nc.tensor.matmul(
                                ps, lhsT=w_all[:, W_Q + kc2, mc * 128:(mc + 1) * 128],
                                rhs=x_sb[:, kc2, nb * 512:(nb + 1) * 512],
                                start=(kc2 == 0), stop=(kc2 == 1))
                        nc.vector.tensor_scalar_add(
                            out=q_sb[:, mc, nb * 512:(nb + 1) * 512], in0=ps,
                            scalar1=qkb_col[:, mc:mc + 1])
                # k = kw @ kv + kb ; vT = kv^T @ vw^T + vb, interleaved per
                # kv chunk so compute starts as soon as each chunk lands
                for nb in range(6):
                    for mc in range(2):
                        ps = proj_ps.tile([128, 512], f32, tag="proj")
                        for kc2 in range(2):
                            nc.tensor.matmul(
                                ps, lhsT=w_all[:, W_K + kc2, mc * 128:(mc + 1) * 128],
                                rhs=kv_sb[:, kc2, nb * 512:(nb + 1) * 512],
                                start=(kc2 == 0), stop=(kc2 == 1))
                        nc.vector.tensor_scalar_add(
                            out=k_sb[:, mc, nb * 512:(nb + 1) * 512], in0=ps,
                            scalar1=qkb_col[:, 2 + mc:3 + mc])
                    for nn in range(4 * nb, 4 * nb + 4):
                        ps = proj_ps.tile([128, C], f32, tag="projv")
                        for kc2 in range(2):
                            nc.tensor.matmul(
                                ps, lhsT=kv_sb[:, kc2, nn * 128:(nn + 1) * 128],
                                rhs=w_all[:, W_V + kc2, :], start=(kc2 == 0),
                                stop=(kc2 == 1))
                        nc.vector.tensor_add(
                            vT_aug[:, nn, :, 0:D],
                            ps.rearrange("p (h e) -> p h e", h=NH),
                            vb_bc.rearrange("p (h e) -> p h e", h=NH))

            # ---- attention + o-proj + LN ----
            NHP = 8          # head-pair units: (qb, hp)
            NU = NHP * 24    # score-chunk units
            VDELAY = 2
            with tc.tile_pool(name="s_ps", bufs=3, space="PSUM") as s_pool, \
                 tc.tile_pool(name="o_ps", bufs=2, space="PSUM") as o_pool, \
                 tc.tile_pool(name="exps", bufs=4) as exp_pool, \
                 tc.tile_pool(name="tails", bufs=2) as tails, \
                 tc.tile_pool(name="fins", bufs=2) as fins:
                po_tiles = {}
                es_tiles = {}

                def emit_scores(u):
                    qb, hp, kc = u // 96, (u // 24) % 4, u % 24
                    hg, sub = hp // 2, hp % 2
                    ps = s_pool.tile([128, 2, 512], f32, tag="S")
                    for j in range(2):
                        pof = 64 * sub + 32 * j
                        nc.tensor.matmul(
                            ps[:, j, :],
                            lhsT=k_sb[pof:pof + 32, hg, kc * 128:(kc + 1) * 128],
                            rhs=q_sb[pof:pof + 32, hg, qb * 512:(qb + 1) * 512],
                            start=True, stop=True, tile_position=(pof, 0))
                    if _use_dve_exp(u):
                        es_i = exp_pool.tile([128, 2, 512], i16, tag="exp")
                        nc.vector.tensor_scalar(
                            out=es_i, in0=ps, scalar1=_SCHR_A16,
                            scalar2=_SCHR_B16, op0=Alu.mult, op1=Alu.add)
                        es_tiles[u] = es_i.bitcast(f16)
                    else:
                        es = exp_pool.tile([128, 2, 512], f16, tag="exp")
                        nc.scalar.activation(es, ps, Act.Exp, scale=SCALE)
                        es_tiles[u] = es

                def emit_av(v):
                    # attn@V for score-chunk v (col-packed heads at 0 / 64)
                    hpi, kc = v // 24, v % 24
                    hp = hpi % 4
                    if kc == 0:
                        po_tiles[hpi] = o_pool.tile([128, 512], f32, tag="opo", name=f"po_{hpi}")
                    po = po_tiles[hpi]
                    es = es_tiles.pop(v)
                    for j in range(2):
                        nc.tensor.matmul(
                            po[64 * j:64 * j + D + 1, :],
                            lhsT=vT_aug[:, kc, hp * 2 + j, :],
                            rhs=es[:, j, :],
                            start=(kc == 0), stop=(kc == 23),
                            tile_position=(0, 64 * j))

                def emit_tail(hpi):
                    # numerators stay put in po layout; only the two
                    # denominator rows (32 / 96) leave PSUM.  Spread the 1024
                    # denominators over 32 partitions via DMA, approx-recip,
                    # broadcast back per head.
                    qb, hp = hpi // 4, hpi % 4
                    po = po_tiles.pop(hpi)
                    raw = tails.tile([128, 512], f32, tag="raw")
                    nc.vector.tensor_copy(raw, po)
                    dp = tails.tile([32, 32], f32, tag="dp")
                    nc.sync.dma_start(out=dp, in_=raw[D:64 + D + 1:64, :])
                    rp = tails.tile([32, 32], f32, tag="rp")
                    nc.vector.reciprocal(rp, dp)
                    rec = tails.tile([1, 2, 512], f32, tag="rec")
                    nc.sync.dma_start(
                        out=rec.rearrange("p a q -> p (a q)"), in_=rp)
                    rbc = tails.tile([128, 512], f32, tag="rbc")
                    for j in range(2):
                        nc.sync.dma_start(out=rbc[64 * j:64 * j + 32, :],
                                          in_=bcast_sbuf_row(rec[0:1, j, :], 32))
                    for j in range(2):
                        nc.vector.tensor_mul(
                            onrm[qb][hp][64 * j:64 * j + 32, :],
                            po[64 * j:64 * j + 32, :],
                            rbc[64 * j:64 * j + 32, :])

                def emit_oproj(qb):
                    # o-projection + residual + bias + LayerNorm per 128 queries
                    for qc2 in range(4):
                        qoff = qb * 512 + qc2 * 128
                        pso = s_pool.tile([128, C], f32, tag="S")
                        for hp in range(4):
                            nc.tensor.matmul(
                                pso, lhsT=onrm[qb][hp][:, qc2 * 128:(qc2 + 1) * 128],
                                rhs=w_all[:, W_OA + hp, :],
                                start=(hp == 0), stop=False)
                        for cc in range(2):
                            nc.tensor.matmul(
                                pso, lhsT=x_sb[:, cc, qoff:qoff + 128],
                                rhs=w_all[:, W_ID + cc, :], start=False, stop=False)
                        nc.tensor.matmul(pso, lhsT=ones_row[0:1, 0:128], rhs=ob_row[:],
                                         start=False, stop=True)
                        stats = fins.tile([128, 6], f32, tag="stats")
                        nc.vector.bn_stats(stats, pso)
                        mv = fins.tile([128, 2], f32, tag="mv")
                        nc.vector.bn_aggr(mv, stats)
                        # rstd = exp(-0.5*ln(var+eps)): stays in the same ACT
                        # table set as the softmax exp (no table reload)
                        lnv = fins.tile([128, 1], f32, tag="lnv")
                        nc.scalar.activation(lnv, mv[:, 1:2], Act.Ln, bias=eps_col[:, 0:1])
                        rstd = fins.tile([128, 1], f32, tag="rstd")
                        nc.scalar.activation(rstd, lnv, Act.Exp, scale=-0.5)
                        t1 = fins.tile([128, C], f32, tag="t1")
                        nc.vector.tensor_scalar(
                            out=t1, in0=pso, scalar1=mv[:, 0:1], scalar2=rstd,
                            op0=Alu.subtract, op1=Alu.mult)
                        # scale/shift on GpSimd: frees DVE cycles for exp
                        t2 = fins.tile([128, C], f32, tag="t2")
                        nc.gpsimd.tensor_mul(t2, t1, lnw_bc)
                        t3 = fins.tile([128, C], f32, tag="t3")
                        nc.gpsimd.tensor_add(t3, t2, lnb_bc)
                        nc.sync.dma_start(out=y_d[qoff:qoff + 128, :], in_=t3)

                for u in range(NU + VDELAY):
                    if u < NU:
                        emit_scores(u)
                    if u >= VDELAY:
                        v = u - VDELAY
                        emit_av(v)
                        if v % 24 == 23:
                            emit_tail(v // 24)
                    if u == 96 + 8:
                        emit_oproj(0)
                emit_oproj(1)
    return nc


_CACHE = {}


def _get_program():
    if "nc" not in _CACHE:
        _apply_walrus_wait_patch()
        _CACHE["nc"] = build_program()
    return _CACHE["nc"]


def _make_in_maps(inputs):
    s3 = np.asarray(inputs["s3"], dtype=np.float32)
    s4 = np.asarray(inputs["s4"], dtype=np.float32)
    s5 = np.asarray(inputs["s5"], dtype=np.float32)
    wts = {}
    for nm in ("qw", "kw", "vw"):
        wts[nm + "T"] = np.asarray(inputs[nm], dtype=np.float32).T.astype(np.float16)
    # o-proj weights permuted + zero-padded to the attn@V PSUM layout:
    # chunk hp rows = [head 2hp (32) | zeros (32) | head 2hp+1 (32) | zeros]
    owT = np.asarray(inputs["ow"], dtype=np.float32).T.astype(np.float16)
    owA = np.zeros((4, 128, C), dtype=np.float16)
    for hp in range(4):
        owA[hp, 0:32] = owT[(2 * hp) * 32:(2 * hp) * 32 + 32]
        owA[hp, 64:96] = owT[(2 * hp + 1) * 32:(2 * hp + 1) * 32 + 32]
    ident = np.eye(C, dtype=np.float16)
    wall = np.ascontiguousarray(np.concatenate(
        [wts["qwT"], wts["kwT"], wts["vwT"], ident, owA.reshape(512, C)], axis=0))
    qkb = np.ascontiguousarray(np.stack(
        [np.asarray(inputs["qb"], np.float32).reshape(2, 128)[0],
         np.asarray(inputs["qb"], np.float32).reshape(2, 128)[1],
         np.asarray(inputs["kb"], np.float32).reshape(2, 128)[0],
         np.asarray(inputs["kb"], np.float32).reshape(2, 128)[1]], axis=1))
    rows3 = np.ascontiguousarray(np.concatenate(
        [np.asarray(inputs["vb"], np.float32).reshape(1, C),
         np.asarray(inputs["ln_w"], np.float32).reshape(1, C),
         np.asarray(inputs["ln_b"], np.float32).reshape(1, C)], axis=1))
    ob_r = np.asarray(inputs["ob"], dtype=np.float32).reshape(1, C).astype(np.float16)
    in_maps = []
    for core in range(N_CORES):
        b, qc = core // 4, core % 4
        x = np.ascontiguousarray(
            s3[b].reshape(C, -1)[:, qc * NQ_CORE:(qc + 1) * NQ_CORE]
        ).astype(np.float16)
        kv = np.concatenate(
            [s4[b].reshape(C, -1), s5[b].reshape(C, -1)], axis=1).astype(np.float16)
        in_maps.append({
            "x": x, "kv": kv, "wall": wall,
            "qkb": qkb, "rows3": rows3, "ob2": ob_r,
        })
    return in_maps


def _assemble(results, like):
    B, _, H, W = 2, C, 64, 64
    out = np.empty((B, C, H * W), dtype=np.float32)
    for core in range(N_CORES):
        b, qc = core // 4, core % 4
        out[b, :, qc * NQ_CORE:(qc + 1) * NQ_CORE] = results[core]["y"].T
    return out.reshape(B, C, H, W)


def kernel(**inputs):
    from concourse import bass2jax
    nc = _get_program()
    in_maps = _make_in_maps(inputs)
    results = bass2jax.run_bass_via_pjrt(nc, in_maps, n_cores=N_CORES)
    return _assemble(results, inputs["s3"])


# revision 11
# speedup vs baseline: 2.6916x; 1.0281x over previous
"""Content-guided attention kernel for Trainium2, 8 NeuronCores SPMD.

Sharding: 8 cores = (batch b in {0,1}) x (query-chunk qc in {0..3}).
Each core computes 1024 query positions of batch b end-to-end:
q/k/vT projections, 8-head attention over all 3072 keys, o-projection,
residual and LayerNorm.  No collectives needed; host splits/concats.

Per-core layout highlights:
 - all matmul operands in fp16 (1 cycle/row on the PE vs 4 for fp32;
   PSUM accumulation stays fp32)
 - scores computed transposed S^T[kpos, qpos] so softmax sum folds into the
   attn@V matmul via a ones-column appended to V^T (no partition reductions)
 - head_dim=32 scores matmuls are packed 4-at-a-time into the PE's 32-row
   groups via tile_position (4x concurrency at K=32)
 - attn@V matmuls col-packed 2-at-a-time (heads at output partitions 0 and
   64 of one PSUM bank) so the M=33 lane waste halves
 - attn@V of score-chunk u issues 2 chunks after its scores, so the exp
   result is always ready when the PE reaches it: the PE stream is
   gap-free, which keeps the HAM clock-gate at full rate (2.4 GHz) instead
   of oscillating back to the cold 1.2 GHz state
 - normalized head outputs stay in the attn@V partition layout; the o-proj
   weight matrix is row-permuted and zero-padded host-side to match, which
   removes the per-head SBUF shuffle DMAs of the numerators
 - o-projection of query block 0 is emitted a few score-chunks into block 1
   so the softmax-normalize tail latency hides behind PE work
 - inputs arrive via few, large DMAs split across both HWDGE queues (SP +
   Activation) because each dma_start costs ~0.65us of trigger time
 - q/k biases folded into the PSUM->SBUF eviction tensor_scalar; v bias via
   tensor_tensor add against a partition-broadcast tile
 - exp split between ScalarE (table exp, fp16 out) and VectorE (Schraudolph
   bit-trick exp emitted as int16 fp16-bitpattern)
 - LayerNorm rstd computed as exp(-0.5*ln(var+eps)) to stay inside the
   single natural_log_exp ACT table set; LN scale/shift run on GpSimd
"""

import numpy as np

C = 256
NH = 8
D = 32
NQ_CORE = 1024
NK = 3072
N_CORES = 8
SCALE = float(D) ** -0.5

# Schraudolph exp constants for an fp16 bit-pattern target:
# bits16 = round(s * SCALE * 1024/ln2 + (15*1024 - 44.56))
_SCHR_A16 = float(np.float32(SCALE * 1024.0 / np.log(2.0)))
_SCHR_B16 = float(np.float32(15.0 * 1024.0 - 44.56))

# exp slots: 3 of every 5 on ScalarE (table exp), 2 of 5 on VectorE
def _use_dve_exp(slot: int) -> bool:
    return slot % 5 >= 3


def _apply_walrus_wait_patch():
    """This walrus build accepts only ONE sync-wait per instruction; split
    extra waits onto single-wait NoOps inserted before the instruction
    (same engine, same block => per-engine program order preserved)."""
    import orjson
    import concourse.bass_utils as bass_utils
    import concourse.bass2jax as bass2jax

    if getattr(bass_utils, "_ant_wait_split_patch", False):
        return
    bass_utils._ant_wait_split_patch = True
    counter = [0]

    def _split_waits(bir_bytes: bytes) -> bytes:
        d = orjson.loads(bir_bytes)
        changed = False

        def process_blocks(blocks):
            nonlocal changed
            for b in blocks:
                insts = b.get("instructions")
                if insts:
                    new = []
                    for ins in insts:
                        si = ins.get("sync_info")
                        waits = si.get("on_wait") if si else None
                        if waits and len(waits) > 1:
                            changed = True
                            for w in waits[:-1]:
                                counter[0] += 1
                                new.append({
                                    "debug": ins.get("debug", 0),
                                    "engine": ins["engine"],
                                    "ins": [],
                                    "outs": [],
                                    "name": f"antwsplit-{counter[0]}",
                                    "opcode": "NoOp",
                                    "sync_info": {"on_wait": [w], "on_update": []},
                                })
                            si["on_wait"] = [waits[-1]]
                        new.append(ins)
                    b["instructions"] = new
                if b.get("blocks"):
                    process_blocks(b["blocks"])

        for f in d.get("functions", []):
            process_blocks(f.get("blocks", []))
        return orjson.dumps(d) if changed else bir_bytes

    orig = bass_utils.compile_bir_kernel

    def compile_bir_kernel(bir, tmpdir, neff_name="file.neff", **kw):
        if isinstance(bir, (bytes, bytearray)):
            bir = _split_waits(bytes(bir))
        elif isinstance(bir, str):
            bir = _split_waits(bir.encode()).decode()
        return orig(bir, tmpdir, neff_name=neff_name, **kw)

    bass_utils.compile_bir_kernel = compile_bir_kernel
    bass2jax.compile_bir_kernel = compile_bir_kernel


def build_program():
    import concourse.bass as bass
    import concourse.tile as tile
    from concourse import mybir

    f32 = mybir.dt.float32
    f16 = mybir.dt.float16
    i16 = mybir.dt.int16
    Alu = mybir.AluOpType
    Act = mybir.ActivationFunctionType

    nc = bass.Bass()

    x_d = nc.dram_tensor("x", [C, NQ_CORE], f16, kind="ExternalInput")
    kv_d = nc.dram_tensor("kv", [C, NK], f16, kind="ExternalInput")
    # all fp16 weights stacked: qwT | kwT | vwT | ident | owA (4x128 rows)
    wall_d = nc.dram_tensor("wall", [12 * 128, C], f16, kind="ExternalInput")
    qkb_d = nc.dram_tensor("qkb", [128, 4], f32, kind="ExternalInput")
    rows3_d = nc.dram_tensor("rows3", [1, 3 * C], f32, kind="ExternalInput")
    ob_d = nc.dram_tensor("ob2", [1, C], f16, kind="ExternalInput")
    y_d = nc.dram_tensor("y", [NQ_CORE, C], f32, kind="ExternalOutput")

    def bcast_part(ap, n):
        # partition-stride-0 view: replicate one partition row across n
        # (DRAM sources only; SBUF partition dims need nonzero step)
        return bass.AP(tensor=ap.tensor, offset=ap.offset,
                       ap=[[0, n]] + [list(a) for a in ap.ap[1:]])

    def bcast_sbuf_row(ap, n):
        # SBUF [1, F] row -> [n, F] DMA source: keep the 1-partition dim,
        # replicate via a step-0 free dim (legal for DMA reads)
        return bass.AP(tensor=ap.tensor, offset=ap.offset,
                       ap=[list(ap.ap[0]), [0, n]] + [list(a) for a in ap.ap[1:]])

    from contextlib import ExitStack
    with tile.TileContext(nc) as tc, ExitStack() as ctx:
            consts = ctx.enter_context(tc.tile_pool(name="consts", bufs=1))
            data = ctx.enter_context(tc.tile_pool(name="data", bufs=1))
            acts = ctx.enter_context(tc.tile_pool(name="acts", bufs=1))
            # ---- inputs: few big DMAs, split across the two HWDGE queues.
            # sync queue: weights + x (unblocks q-proj first);
            # scalar queue: kv chunks + small consts.
            w_all = consts.tile([128, 12, C], f16, tag="w_all")
            nc.sync.dma_start(out=w_all,
                              in_=wall_d.rearrange("(a p) c -> p a c", p=128))
            W_Q, W_K, W_V, W_ID, W_OA = 0, 2, 4, 6, 8

            x_sb = data.tile([128, 2, NQ_CORE], f16, tag="x_sb")
            x_r = x_d.rearrange("(a p) n -> p a n", p=128)
            for nb in range(2):
                nc.sync.dma_start(out=x_sb[:, :, nb * 512:(nb + 1) * 512],
                                  in_=x_r[:, :, nb * 512:(nb + 1) * 512])
            kv_sb = data.tile([128, 2, NK], f16, tag="kv_sb")
            kv_r = kv_d.rearrange("(a p) n -> p a n", p=128)
            for nb in range(6):
                nc.scalar.dma_start(out=kv_sb[:, :, nb * 512:(nb + 1) * 512],
                                    in_=kv_r[:, :, nb * 512:(nb + 1) * 512])
            qkb_col = consts.tile([128, 4], f32, tag="qkb_col")
            nc.scalar.dma_start(out=qkb_col, in_=qkb_d[:])
            rows3_bc = consts.tile([128, 3 * C], f32, tag="rows3_bc")
            nc.scalar.dma_start(out=rows3_bc, in_=bcast_part(rows3_d[:], 128))
            vb_bc = rows3_bc[:, 0:C]
            lnw_bc = rows3_bc[:, C:2 * C]
            lnb_bc = rows3_bc[:, 2 * C:3 * C]
            ob_row = consts.tile([1, C], f16, tag="ob_row")
            nc.scalar.dma_start(out=ob_row, in_=ob_d[:])
            ones_row = consts.tile([1, 512], f16, tag="ones_row")
            nc.vector.memset(ones_row, 1.0)
            eps_col = consts.tile([128, 1], f32, tag="eps_col")
            nc.vector.memset(eps_col, 1e-5)

            q_sb = acts.tile([128, 2, NQ_CORE], f16, tag="q_sb")
            k_sb = acts.tile([128, 2, NK], f16, tag="k_sb")
            vT_aug = acts.tile([128, 24, NH, D + 1], f16, tag="vT_aug")
            nc.vector.memset(vT_aug[:, :, :, D:D + 1], 1.0)
            # onrm[qb][hp]: normalized attn@V numerators in po layout
            # (head A rows 0:32, head B rows 64:96).  Rows 32:64 / 96:128 are
            # dead lanes multiplied by zero-padded owA rows in the o-proj;
            # memset once so stale SBUF NaNs can't propagate through 0*x.
            onrm = [[acts.tile([128, 512], f16, tag=f"onrm_{qb}_{hp}",
                                name=f"onrm_{qb}_{hp}")
                     for hp in range(4)] for qb in range(2)]
            for qb in range(2):
                for hp in range(4):
                    nc.vector.memset(onrm[qb][hp][32:64, :], 0.0)
                    nc.vector.memset(onrm[qb][hp][96:128, :], 0.0)

            # ---- projections (bias folded into the PSUM->SBUF eviction) ----
            with tc.tile_pool(name="proj_ps", bufs=4, space="PSUM") as proj_ps:
                # q = qw @ x + qb   (chunks of output channels x 512 cols)
                for mc in range(2):
                    for nb in range(2):
                        ps = proj_ps.tile([128, 512], f32, tag="proj")
                        for kc2 in range(2):
                            nc.tensor.matmul(
                                ps, lhsT=w_all[:, W_Q + kc2, mc * 128:(mc + 1) * 128],
                                rhs=x_sb[:, kc2, nb * 512:(nb + 1) * 512],
                                start=(kc2 == 0), stop=(kc2 == 1))
                        nc.vector.tensor_scalar_add(
                            out=q_sb[:, mc, nb * 512:(nb + 1) * 512], in0=ps,
                            scalar1=qkb_col[:, mc:mc + 1])
                # k = kw @ kv + kb ; vT = kv^T @ vw^T + vb, interleaved per
                # kv chunk so compute starts as soon as each chunk lands
                for nb in range(6):
                    for mc in range(2):
                        ps = proj_ps.tile([128, 512], f32, tag="proj")
                        for kc2 in range(2):
                            nc.tensor.matmul(
                                ps, lhsT=w_all[:, W_K + kc2, mc * 128:(mc + 1) * 128],
                                rhs=kv_sb[:, kc2, nb * 512:(nb + 1) * 512],
                                start=(kc2 == 0), stop=(kc2 == 1))
                        nc.vector.tensor_scalar_add(
                            out=k_sb[:, mc, nb * 512:(nb + 1) * 512], in0=ps,
                            scalar1=qkb_col[:, 2 + mc:3 + mc])
                    for nn in range(4 * nb, 4 * nb + 4):
                        ps = proj_ps.tile([128, C], f32, tag="projv")
                        for kc2 in range(2):
                            nc.tensor.matmul(
                                ps, lhsT=kv_sb[:, kc2, nn * 128:(nn + 1) * 128],
                                rhs=w_all[:, W_V + kc2, :], start=(kc2 == 0),
                                stop=(kc2 == 1))
                        nc.vector.tensor_add(
                            vT_aug[:, nn, :, 0:D],
                            ps.rearrange("p (h e) -> p h e", h=NH),
                            vb_bc.rearrange("p (h e) -> p h e", h=NH))

            # ---- attention + o-proj + LN ----
            NHP = 8          # head-pair units: (qb, hp)
            NU = NHP * 24    # score-chunk units
            VDELAY = 2
            with tc.tile_pool(name="s_ps", bufs=3, space="PSUM") as s_pool, \
                 tc.tile_pool(name="o_ps", bufs=2, space="PSUM") as o_pool, \
                 tc.tile_pool(name="exps", bufs=4) as exp_pool, \
                 tc.tile_pool(name="tails", bufs=2) as tails, \
                 tc.tile_pool(name="fins", bufs=4) as fins:
                po_tiles = {}
                es_tiles = {}

                def emit_scores(u):
                    qb, hp, kc = u // 96, (u // 24) % 4, u % 24
                    hg, sub = hp // 2, hp % 2
                    ps = s_pool.tile([128, 2, 512], f32, tag="S")
                    for j in range(2):
                        pof = 64 * sub + 32 * j
                        nc.tensor.matmul(
                            ps[:, j, :],
                            lhsT=k_sb[pof:pof + 32, hg, kc * 128:(kc + 1) * 128],
                            rhs=q_sb[pof:pof + 32, hg, qb * 512:(qb + 1) * 512],
                            start=True, stop=True, tile_position=(pof, 0))
                    if _use_dve_exp(u):
                        es_i = exp_pool.tile([128, 2, 512], i16, tag="exp")
                        nc.vector.tensor_scalar(
                            out=es_i, in0=ps, scalar1=_SCHR_A16,
                            scalar2=_SCHR_B16, op0=Alu.mult, op1=Alu.add)
                        es_tiles[u] = es_i.bitcast(f16)
                    else:
                        es = exp_pool.tile([128, 2, 512], f16, tag="exp")
                        nc.scalar.activation(es, ps, Act.Exp, scale=SCALE)
                        es_tiles[u] = es

                def emit_av(v):
                    # attn@V for score-chunk v (col-packed heads at 0 / 64)
                    hpi, kc = v // 24, v % 24
                    hp = hpi % 4
                    if kc == 0:
                        po_tiles[hpi] = o_pool.tile([128, 512], f32, tag="opo", name=f"po_{hpi}")
                    po = po_tiles[hpi]
                    es = es_tiles.pop(v)
                    for j in range(2):
                        nc.tensor.matmul(
                            po[64 * j:64 * j + D + 1, :],
                            lhsT=vT_aug[:, kc, hp * 2 + j, :],
                            rhs=es[:, j, :],
                            start=(kc == 0), stop=(kc == 23),
                            tile_position=(0, 64 * j))

                def emit_tail(hpi):
                    # numerators stay put in po layout; only the two
                    # denominator rows (32 / 96) leave PSUM.  Spread the 1024
                    # denominators over 32 partitions via DMA, approx-recip,
                    # broadcast back per head.
                    qb, hp = hpi // 4, hpi % 4
                    po = po_tiles.pop(hpi)
                    raw = tails.tile([128, 512], f32, tag="raw")
                    nc.vector.tensor_copy(raw, po)
                    dp = tails.tile([32, 32], f32, tag="dp")
                    nc.sync.dma_start(out=dp, in_=raw[D:64 + D + 1:64, :])
                    rp = tails.tile([32, 32], f16, tag="rp")
                    with nc.allow_low_precision(reason="softmax denom recip in fp16"):
                        nc.vector.reciprocal(rp, dp)
                    rec = tails.tile([1, 2, 512], f16, tag="rec")
                    nc.sync.dma_start(
                        out=rec.rearrange("p a q -> p (a q)"), in_=rp)
                    rbc = tails.tile([128, 512], f16, tag="rbc")
                    nc.sync.dma_start(out=rbc[0:32, :],
                                      in_=bcast_sbuf_row(rec[0:1, 0, :], 32))
                    nc.scalar.dma_start(out=rbc[64:96, :],
                                        in_=bcast_sbuf_row(rec[0:1, 1, :], 32))
                    for j in range(2):
                        nc.vector.tensor_mul(
                            onrm[qb][hp][64 * j:64 * j + 32, :],
                            po[64 * j:64 * j + 32, :],
                            rbc[64 * j:64 * j + 32, :])

                def emit_oproj(qb):
                    # o-projection + residual + bias + LayerNorm per 128 queries
                    for qc2 in range(4):
                        qoff = qb * 512 + qc2 * 128
                        pso = s_pool.tile([128, C], f32, tag="S")
                        for hp in range(4):
                            nc.tensor.matmul(
                                pso, lhsT=onrm[qb][hp][:, qc2 * 128:(qc2 + 1) * 128],
                                rhs=w_all[:, W_OA + hp, :],
                                start=(hp == 0), stop=False)
                        for cc in range(2):
                            nc.tensor.matmul(
                                pso, lhsT=x_sb[:, cc, qoff:qoff + 128],
                                rhs=w_all[:, W_ID + cc, :], start=False, stop=False)
                        nc.tensor.matmul(pso, lhsT=ones_row[0:1, 0:128], rhs=ob_row[:],
                                         start=False, stop=True)
                        stats = fins.tile([128, 6], f32, tag="stats")
                        nc.vector.bn_stats(stats, pso)
                        mv = fins.tile([128, 2], f32, tag="mv")
                        nc.vector.bn_aggr(mv, stats)
                        # rstd = exp(-0.5*ln(var+eps)): stays in the same ACT
                        # table set as the softmax exp (no table reload)
                        lnv = fins.tile([128, 1], f32, tag="lnv")
                        nc.scalar.activation(lnv, mv[:, 1:2], Act.Ln, bias=eps_col[:, 0:1])
                        rstd = fins.tile([128, 1], f32, tag="rstd")
                        nc.scalar.activation(rstd, lnv, Act.Exp, scale=-0.5)
                        t1 = fins.tile([128, C], f32, tag="t1")
                        nc.vector.tensor_scalar(
                            out=t1, in0=pso, scalar1=mv[:, 0:1], scalar2=rstd,
                            op0=Alu.subtract, op1=Alu.mult)
                        # scale/shift on GpSimd: frees DVE cycles for exp
                        eng = nc.gpsimd if qb == 0 else nc.vector
                        t2 = fins.tile([128, C], f32, tag="t2")
                        eng.tensor_mul(t2, t1, lnw_bc)
                        t3 = fins.tile([128, C], f32, tag="t3")
                        eng.tensor_add(t3, t2, lnb_bc)
                        nc.sync.dma_start(out=y_d[qoff:qoff + 128, :], in_=t3)

                for u in range(NU + VDELAY):
                    if u < NU:
                        emit_scores(u)
                    if u >= VDELAY:
                        v = u - VDELAY
                        emit_av(v)
                        if v % 24 == 23:
                            emit_tail(v // 24)
                    if u == 96 + 8:
                        emit_oproj(0)
                emit_oproj(1)
    return nc


_CACHE = {}


def _get_program():
    if "nc" not in _CACHE:
        _apply_walrus_wait_patch()
        _CACHE["nc"] = build_program()
    return _CACHE["nc"]


def _make_in_maps(inputs):
    s3 = np.asarray(inputs["s3"], dtype=np.float32)
    s4 = np.asarray(inputs["s4"], dtype=np.float32)
    s5 = np.asarray(inputs["s5"], dtype=np.float32)
    wts = {}
    for nm in ("qw", "kw", "vw"):
        wts[nm + "T"] = np.asarray(inputs[nm], dtype=np.float32).T.astype(np.float16)
    # o-proj weights permuted + zero-padded to the attn@V PSUM layout:
    # chunk hp rows = [head 2hp (32) | zeros (32) | head 2hp+1 (32) | zeros]
    owT = np.asarray(inputs["ow"], dtype=np.float32).T.astype(np.float16)
    owA = np.zeros((4, 128, C), dtype=np.float16)
    for hp in range(4):
        owA[hp, 0:32] = owT[(2 * hp) * 32:(2 * hp) * 32 + 32]
        owA[hp, 64:96] = owT[(2 * hp + 1) * 32:(2 * hp + 1) * 32 + 32]
    ident = np.eye(C, dtype=np.float16)
    wall = np.ascontiguousarray(np.concatenate(
        [wts["qwT"], wts["kwT"], wts["vwT"], ident, owA.reshape(512, C)], axis=0))
    qkb = np.ascontiguousarray(np.stack(
        [np.asarray(inputs["qb"], np.float32).reshape(2, 128)[0],
         np.asarray(inputs["qb"], np.float32).reshape(2, 128)[1],
         np.asarray(inputs["kb"], np.float32).reshape(2, 128)[0],
         np.asarray(inputs["kb"], np.float32).reshape(2, 128)[1]], axis=1))
    rows3 = np.ascontiguousarray(np.concatenate(
        [np.asarray(inputs["vb"], np.float32).reshape(1, C),
         np.asarray(inputs["ln_w"], np.float32).reshape(1, C),
         np.asarray(inputs["ln_b"], np.float32).reshape(1, C)], axis=1))
    ob_r = np.asarray(inputs["ob"], dtype=np.float32).reshape(1, C).astype(np.float16)
    in_maps = []
    for core in range(N_CORES):
        b, qc = core // 4, core % 4
        x = np.ascontiguousarray(
            s3[b].reshape(C, -1)[:, qc * NQ_CORE:(qc + 1) * NQ_CORE]
        ).astype(np.float16)
        kv = np.concatenate(
            [s4[b].reshape(C, -1), s5[b].reshape(C, -1)], axis=1).astype(np.float16)
        in_maps.append({
            "x": x, "kv": kv, "wall": wall,
            "qkb": qkb, "rows3": rows3, "ob2": ob_r,
        })
    return in_maps


def _assemble(results, like):
    B, _, H, W = 2, C, 64, 64
    out = np.empty((B, C, H * W), dtype=np.float32)
    for core in range(N_CORES):
        b, qc = core // 4, core % 4
        out[b, :, qc * NQ_CORE:(qc + 1) * NQ_CORE] = results[core]["y"].T
    return out.reshape(B, C, H, W)


def kernel(**inputs):
    from concourse import bass2jax
    nc = _get_program()
    in_maps = _make_in_maps(inputs)
    results = bass2jax.run_bass_via_pjrt(nc, in_maps, n_cores=N_CORES)
    return _assemble(results, inputs["s3"])
